# revision 59
# baseline (speedup 1.0000x reference)
"""ALiBi sliding-window GQA attention on 8 Trainium2 NeuronCores.

Sharding: batch (2) x sequence quarter (4) -> 8 cores, each computing a
disjoint [512, 1024] output chunk from a 528-token input slice (16-token
halo on the left for the sliding window). No collectives needed.

Per-core kernel (bf16 compute, f32 accumulate), v3 dataflow:
  1. K projection runs weights-stationary so K^T materializes directly in
     [feature, token] layout (no PE transposes), accumulating per kt-tile
     as the xt DMA chunks stream in -- the PE starts ~5us earlier than a
     monolithic load allows.  K is NOT normalized in SBUF: its RMSNorm
     rstd (x0.125 score scale) is applied per-key-partition via the exp
     activation's scale vector, and q_norm_w*k_norm_w is folded into the
     PSUM->SBUF eviction scale.  Per-head sum-of-squares stats come from
     a block-diagonal ones matmul on the squared K^T.
  2. Q projection in [token, feature] layout (stats need free-axis
     reduction), RMSNorm'd, then transposed to [feature, token] via PE
     transpose-mode.
  3. 5 query blocks of 112 with 128-key windows; per kv-group scores come
     out PRE-TRANSPOSED: S.T[key, head-slot, query] = K^T.T @ Q^T.
       P.T = exp(rstd_k[key] * S.T) * ebT (host table: ALiBi slopes +
       causal/window mask as multiplicative zeros, pre-transposed,
       slot-permuted)
     row sums via ones-matmul, paired so one full-width reciprocal and
     one full-width normalize serve two kv-groups; normalization fused
     into the AV PSUM->SBUF eviction.  exp on Scalar, ebT-mult split
     GpSimd/Vector.
  4. Output projection uses OUT^T as the stationary operand so results
     land in [token, feature] layout for contiguous stores.
"""

import math

import numpy as np
import ml_dtypes

import concourse.bass as bass
import concourse.tile as tile
from concourse import bacc, mybir
from concourse.bass_utils import run_bass_kernel_spmd

BF16 = ml_dtypes.bfloat16

B, L, DIM = 2, 2048, 1024
N_HEADS, N_KV_HEADS, HEAD_DIM = 16, 4, 64
WINDOW = 16
EPS = 0.01

LQ = 512           # queries per core
HALO = WINDOW      # left halo
NB = 5             # query blocks per core (attention)
BQ = 112           # queries per block
BK = 128           # key window per block
LX = HALO + NB * BQ  # 576 = padded x slice width per core
LH = LX // 2       # 288: K-proj moving-operand half
P = 128
QC = 4             # Q/O projection chunks of 128 tokens (4*128 = LQ exactly)

# head h = g + 4j (g = kv group) -> scores slot 4g + j: each attention step
# batches the 4 heads of one kv group into a single matmul
SLOT = [4 * (h % 4) + h // 4 for h in range(N_HEADS)]

_compiled = None


def _alibi_slopes(n_heads):
    closest = 2 ** math.floor(math.log2(n_heads))
    base = 2.0 ** (-(2.0 ** (-(math.log2(closest) - 3))))
    slopes = base ** np.arange(1, closest + 1, dtype=np.float64)
    if closest < n_heads:
        eb = 2.0 ** (-(2.0 ** (-(math.log2(2 * closest) - 3))))
        extra = eb ** np.arange(1, 2 * (n_heads - closest) + 1, 2, dtype=np.float64)
        slopes = np.concatenate([slopes, extra])
    return slopes[:n_heads]


def _exp_bias_t(edge: bool) -> np.ndarray:
    """[BK, N_HEADS, BQ] transposed multiplicative softmax bias, slot order.

    Query i (block-local) sits at window column jk in [i, i+16]; entry is
    exp(slope_h * (jk - 16 - i)) inside the band, 0 outside.  With
    edge=True (first block of the sequence) keys at global position < 0
    (jk < 16) are additionally masked.
    """
    slopes = _alibi_slopes(N_HEADS)
    i = np.arange(BQ)[:, None]
    jk = np.arange(BK)[None, :]
    rel = jk - WINDOW - i                      # [BQ, BK]
    valid = (rel <= 0) & (rel >= -WINDOW)
    if edge:
        valid = valid & (jk >= WINDOW)
    arg = np.where(valid[None], slopes[:, None, None] * rel[None], -np.inf)
    eb = np.exp(arg)                           # [H, BQ, BK]
    perm = np.empty(N_HEADS, np.int64)
    perm[SLOT] = np.arange(N_HEADS)            # slot s holds head perm[s]
    return np.ascontiguousarray(eb[perm].transpose(2, 0, 1)).astype(BF16)


def _build():
    nc = bacc.Bacc("TRN2", target_bir_lowering=False, debug=False)
    f32, bf16 = mybir.dt.float32, mybir.dt.bfloat16

    xt_e = nc.dram_tensor("xt", [P, 8, LX], bf16, kind="ExternalInput")
    wq_e = nc.dram_tensor("wqT", [P, 8, 1024], bf16, kind="ExternalInput")
    wk_e = nc.dram_tensor("wkT", [P, 8, 256], bf16, kind="ExternalInput")
    wv_e = nc.dram_tensor("wvT", [P, 8, 256], bf16, kind="ExternalInput")
    wo_e = nc.dram_tensor("woT", [P, 8, 1024], bf16, kind="ExternalInput")
    eb0_e = nc.dram_tensor("ebT0", [BK, N_HEADS, BQ], bf16, kind="ExternalInput")
    ebr_e = nc.dram_tensor("ebTr", [BK, N_HEADS, BQ], bf16, kind="ExternalInput")
    u_e = nc.dram_tensor("uvec", [P, 1], f32, kind="ExternalInput")
    ob2_e = nc.dram_tensor("oblk2", [2, P], bf16, kind="ExternalInput")
    out_e = nc.dram_tensor("out", [LQ, DIM], bf16, kind="ExternalOutput")

    NQ = 2 * QC  # 8 Q-chunk tiles

    with tile.TileContext(nc) as tc:
        with (
            tc.tile_pool(name="w", bufs=1) as wp,
            tc.tile_pool(name="glob", bufs=1) as gp,
            tc.tile_pool(name="raw", bufs=NQ) as rp,
            tc.tile_pool(name="stage", bufs=4) as sp,
            tc.tile_pool(name="small", bufs=NQ) as mp,
            tc.tile_pool(name="att", bufs=3) as ap,
            tc.tile_pool(name="ptrs", bufs=2 * NB) as pt,
            tc.tile_pool(name="vpool", bufs=NB) as vp,
            tc.tile_pool(name="pp", bufs=2, space="PSUM") as pp,
        ):
            # ---- PE warmup: junk matmuls bridge the DMA-bound prologue so
            # the HAM clock-gate opens before real work arrives ----
            junk = wp.tile([P, 512], bf16)
            nc.vector.memset(junk[:], 1.0)
            ones64 = wp.tile([P, 64], bf16)
            nc.vector.memset(ones64[:], 1.0)
            epsv = wp.tile([P, 1], f32)
            nc.vector.memset(epsv[:], EPS)
            eps64v = wp.tile([P, 1], f32)
            nc.vector.memset(eps64v[:], HEAD_DIM * EPS)
            # block-diagonal ones: col j = 1 on partitions [64j, 64j+64)
            ones_blk = wp.tile([P, 2], bf16)
            nc.vector.memset(ones_blk[:], 0.0)
            nc.vector.memset(ones_blk[:64, 0:1], 1.0)
            nc.vector.memset(ones_blk[64:, 1:2], 1.0)
            wps = pp.tile([P, 512], f32, tag="pp", name="warm")[:64]
            for _ in range(10):
                nc.tensor.matmul(wps, ones64[:], junk[:], start=True, stop=True)

            # ---- input loads; wk first, then xt kt-pair by kt-pair so the
            # K projection starts accumulating as chunks land ----
            xt = wp.tile([P, 8, LX], bf16)
            wkT = wp.tile([P, 8, 256], bf16)
            wqT = wp.tile([P, 8, 1024], bf16)
            wvT = wp.tile([P, 8, 256], bf16)
            woT = wp.tile([P, 8, 1024], bf16)
            uvec = wp.tile([P, 1], f32)
            ones_blk2 = wp.tile([2, P], bf16)
            ebT0 = wp.tile([BK, N_HEADS, BQ], bf16)
            ebTr = wp.tile([BK, N_HEADS, BQ], bf16)
            nc.sync.dma_start(wkT[:], wk_e.ap())
            for g in range(4):
                nc.sync.dma_start(xt[:, 2 * g:2 * g + 2],
                                  xt_e.ap()[:, 2 * g:2 * g + 2])
            nc.sync.dma_start(uvec[:], u_e.ap())
            nc.sync.dma_start(ones_blk2[:], ob2_e.ap())
            for g in range(4):
                nc.sync.dma_start(wqT[:, 2 * g:2 * g + 2],
                                  wq_e.ap()[:, 2 * g:2 * g + 2])
            nc.sync.dma_start(wvT[:], wv_e.ap())
            nc.sync.dma_start(ebT0[:], eb0_e.ap())
            nc.sync.dma_start(ebTr[:], ebr_e.ap())
            for g in range(4):
                nc.sync.dma_start(woT[:, 2 * g:2 * g + 2],
                                  wo_e.ap()[:, 2 * g:2 * g + 2])

            QT = gp.tile([P, 8, LQ], bf16)    # normalized Q transposed
            KT = gp.tile([P, 2, LX], bf16)    # raw K^T, u-scaled
            sqKT = gp.tile([P, 2, LX], bf16)  # squared raw K^T (stats)
            OT = gp.tile([P, 8, LQ], bf16)    # attention out transposed

            vbs = []
            rks = []
            with (
                tc.tile_pool(name="kacc", bufs=4, space="PSUM") as ka,
                tc.tile_pool(name="kst", bufs=2, space="PSUM") as ks,
            ):
                # ---- phase 1a: K projection, weights-stationary ----
                kaccs = [ka.tile([P, 512], f32, tag="ka", name="kacc")[:, :LH]
                         for _ in range(4)]
                for kt in range(8):
                    for fh in range(2):
                        for lh in range(2):
                            nc.tensor.matmul(
                                kaccs[2 * fh + lh],
                                wkT[:, kt, fh * P:(fh + 1) * P],
                                xt[:, kt, lh * LH:(lh + 1) * LH],
                                start=(kt == 0), stop=(kt == 7),
                            )
                for fh in range(2):
                    for lh in range(2):
                        acc = kaccs[2 * fh + lh]
                        nc.scalar.activation(
                            KT[:, fh, lh * LH:(lh + 1) * LH], acc[:],
                            mybir.ActivationFunctionType.Copy, scale=uvec[:],
                        )
                        nc.scalar.square(
                            sqKT[:, fh, lh * LH:(lh + 1) * LH], acc[:])

                # ---- phase 1c: K RMSNorm stats per key window ----
                # rk = 0.125 / sqrt(mean(k^2) + eps) = 1/sqrt(sumsq + 64*eps)
                # per (window key partition, kv-head); consumed as the exp
                # activation's per-partition scale vector.
                for b_ in range(NB):
                    qs = b_ * BQ
                    kss = ks.tile([P, 4], f32, tag="ks", name="kss")
                    for fh in range(2):
                        nc.tensor.matmul(
                            kss[:, 2 * fh:2 * fh + 2],
                            sqKT[:, fh, qs:qs + BK],
                            ones_blk[:],
                            start=True, stop=True,
                        )
                    srtk = mp.tile([P, 8], f32, tag="srt", name="srtk")[:, :4]
                    nc.scalar.activation(
                        srtk[:], kss[:], mybir.ActivationFunctionType.Sqrt,
                        scale=1.0, bias=eps64v[:],
                    )
                    rk = mp.tile([P, 4], f32, tag="rk", name="rk", bufs=NB)
                    nc.vector.reciprocal(rk[:], srtk[:])
                    rks.append(rk)

            # ---- phase 2: Q projection, weights-stationary -> QT slot fc
            # directly (fc = 128-feature chunk = QT slot = heads 2fc,2fc+1).
            # Per-head stats via the same block-diagonal-ones matmul as K;
            # rstd replicated across each head's 64 partitions by a tiny
            # contraction-2 matmul, then one Vector multiply lands the
            # normalized QT slot.  No PE transposes, no free-axis reduces.
            qraws, sqs, rstds = [], [], []
            with (
                tc.tile_pool(name="qst", bufs=2, space="PSUM") as qsp,
                tc.tile_pool(name="qrp", bufs=2, space="PSUM") as qrp,
            ):
                def emit_qchain(fc):
                    q_ps = pp.tile([P, 512], f32, tag="pp", name="q_ps")
                    for kt in range(8):
                        nc.tensor.matmul(
                            q_ps,
                            wqT[:, kt, fc * P:(fc + 1) * P],
                            xt[:, kt, HALO:HALO + LQ],
                            start=(kt == 0), stop=(kt == 7),
                        )
                    qraw = rp.tile([P, 512], bf16, tag="raw", name="qraw")
                    nc.scalar.copy(qraw[:], q_ps[:])
                    sq = sp.tile([P, 512], bf16, tag="sq", name="sq")
                    nc.scalar.square(sq[:], q_ps[:])
                    qraws.append(qraw)
                    sqs.append(sq)

                def emit_qstats(fc):
                    qst = qsp.tile([2, 512], f32, tag="qst", name="qst")
                    nc.tensor.matmul(qst[:], ones_blk[:], sqs[fc][:],
                                     start=True, stop=True)
                    srt = mp.tile([2, 512], f32, tag="qsrt", name="qsrt",
                                  bufs=2)
                    nc.scalar.activation(
                        srt[:], qst[:], mybir.ActivationFunctionType.Sqrt,
                        scale=1.0 / HEAD_DIM, bias=epsv[:2],
                    )
                    rstd = mp.tile([2, 512], bf16, tag="qrstd", name="qrstd",
                                   bufs=2)
                    with nc.allow_low_precision(
                            reason="bf16 rstd feeds a bf16 replicate matmul"):
                        nc.vector.reciprocal(rstd[:], srt[:])
                    rstds.append(rstd)

                def emit_qrep(fc):
                    rep = qrp.tile([P, 512], f32, tag="qrep", name="qrep")
                    nc.tensor.matmul(rep[:], ones_blk2[:], rstds[fc][:],
                                     start=True, stop=True)
                    nc.vector.tensor_tensor(
                        QT[:, fc, :], rep[:], qraws[fc][:],
                        mybir.AluOpType.mult,
                    )

                for fc in range(8):
                    emit_qchain(fc)
                    if fc >= 1:
                        emit_qstats(fc - 1)
                    if fc >= 2:
                        emit_qrep(fc - 2)
                emit_qstats(7)
                emit_qrep(6)
                emit_qrep(7)

                # dummy exp: forces the Exp<->Sqrt ACT-table swap to happen
                # here instead of on the first real exp's critical path
                dume = mp.tile([P, 8], f32, tag="dume", name="dume")
                nc.scalar.activation(
                    dume[:, :1], epsv[:], mybir.ActivationFunctionType.Exp)

            with (
                tc.tile_pool(name="pot", bufs=2, space="PSUM") as pot,
                tc.tile_pool(name="psc", bufs=4, space="PSUM") as psc,
            ):
                # ---- phase 4: attention + output projection ----
                # Flat software-pipelined stream over NB*4 kv-group steps.
                # All 4 heads of kv-group g share the same K/V stationary
                # operand, so each stage is ONE matmul with the 4 heads
                # batched in the moving operand (448 cols).  Head h = g + 4j
                # lives at score-slot (g, j); packed flat [4W] in PSUM.  The
                # PE queue runs scores(i+2) between scores(i) and AV(i) so
                # the exp -> ebT-mult chain latency is hidden by real matmul
                # work even across block boundaries; O-projection chunks
                # slot in at block boundaries as extra filler.
                blocks = []
                for b_ in range(NB):
                    blocks.append(dict(
                        qs=b_ * BQ,
                        W=BQ if b_ < NB - 1 else LQ - (NB - 1) * BQ,
                        ebT=ebT0 if b_ == 0 else ebTr,
                        rk=rks[b_],
                        ot_g={}, rcps={}, dens={}, scs={}, ptrs={},
                    ))

                def emit_vproj(b_):
                    # V projection for block b_'s key window, deferred into
                    # the attention stream as PE filler (vb is first needed
                    # at block b_'s AV step)
                    qs = b_ * BQ
                    v_ps = pp.tile([P, 512], f32, tag="pp", name="v_ps")[:, :256]
                    for kt in range(8):
                        nc.tensor.matmul(
                            v_ps, xt[:, kt, qs:qs + BK], wvT[:, kt],
                            start=(kt == 0), stop=(kt == 7),
                        )
                    vb = vp.tile([P, 256], bf16, tag="vb", name="vb")
                    nc.scalar.copy(vb[:], v_ps[:])
                    vbs.append(vb)

                def emit_scores(b_, g):
                    st = blocks[b_]
                    qs, W = st["qs"], st["W"]
                    sc = psc.tile([P, 4 * BQ], f32, tag="sc", name="sc")
                    # moving: the 4 heads {g+4j} = OT-slots g//2 + 2j of QT
                    mv = QT[:].rearrange("p (j s) l -> p s j l", s=2)[
                        (g % 2) * 64:(g % 2) * 64 + 64, g // 2, :, qs:qs + W]
                    nc.tensor.matmul(
                        sc[:, :4 * W],
                        KT[(g % 2) * 64:(g % 2) * 64 + 64, g // 2, qs:qs + BK],
                        mv,
                        start=True, stop=True,
                    )
                    st["scs"][g] = sc

                def emit_expmult(b_, g):
                    st = blocks[b_]
                    W = st["W"]
                    e_t = ap.tile([P, 4 * BQ], bf16, tag="et", name="e_t")
                    nc.scalar.activation(
                        e_t[:, :4 * W], st["scs"][g][:, :4 * W],
                        mybir.ActivationFunctionType.Exp,
                        scale=st["rk"][:, g:g + 1],
                    )
                    # mask+ALiBi multiply split across GpSimd (slots 0-1)
                    # and Vector (slots 2-3)
                    ptr = pt.tile([P, 4 * BQ], bf16, tag="ptr", name="ptr")
                    nc.gpsimd.tensor_tensor(
                        ptr[:, :2 * W].rearrange("p (s i) -> p s i", i=W),
                        e_t[:, :2 * W].rearrange("p (s i) -> p s i", i=W),
                        st["ebT"][:, 4 * g:4 * g + 2, :W],
                        mybir.AluOpType.mult,
                    )
                    nc.vector.tensor_tensor(
                        ptr[:, 2 * W:4 * W].rearrange("p (s i) -> p s i", i=W),
                        e_t[:, 2 * W:4 * W].rearrange("p (s i) -> p s i", i=W),
                        st["ebT"][:, 4 * g + 2:4 * g + 4, :W],
                        mybir.AluOpType.mult,
                    )
                    st["ptrs"][g] = ptr

                def emit_avden(b_, g):
                    # den for the kv-group pair (grp = g//2) lands in one
                    # [128, 4W] tile -- g even in rows 0-63, g odd in
                    # 64-127 -- so ONE full-width reciprocal and ONE
                    # full-width normalize serve both groups (DVE time
                    # scales with elems/partition, not partitions).
                    st = blocks[b_]
                    W = st["W"]
                    grp = g // 2
                    if grp not in st["ot_g"]:
                        st["ot_g"][grp] = pot.tile([P, 4 * BQ], f32, tag="ot",
                                                   name="ot_g")
                    ptr = st["ptrs"][g]
                    hb = (g % 2) * 64
                    nc.tensor.matmul(
                        st["ot_g"][grp][hb:hb + 64, :4 * W],
                        vbs[b_][:, g * 64:(g + 1) * 64],
                        ptr[:, :4 * W],
                        start=True, stop=True,
                    )
                    if g % 2 == 0:
                        st["dens"][grp] = psc.tile([P, 4 * BQ], f32, tag="sc",
                                                   name="den")
                    den = st["dens"][grp]
                    nc.tensor.matmul(
                        den[hb:hb + 64, :4 * W], ones64[:], ptr[:, :4 * W],
                        start=True, stop=True,
                    )
                    if g % 2 == 1:
                        rcp = sp.tile([P, 4 * BQ], f32, tag="rcp", name="rcp")
                        nc.vector.reciprocal_approx_fast(
                            rcp[:, :4 * W], den[:, :4 * W])
                        st["rcps"][grp] = rcp

                def emit_norm(b_, grp, half=None):
                    # grp 0 = kv-groups 0,1 (OT slots 0,2,4,6); grp 1 = 2,3.
                    # half=0/1 emits only the lower/upper two head-slots (the
                    # last block splits so the final O-proj chain can start
                    # after the first half).
                    st = blocks[b_]
                    qs, W = st["qs"], st["W"]
                    j0, j1 = (0, 4) if half is None else (2 * half, 2 * half + 2)
                    nc.vector.tensor_tensor(
                        OT[:].rearrange("p (j s) l -> p s j l", s=2)[
                            :, grp, j0:j1, qs:qs + W],
                        st["ot_g"][grp][:, j0 * W:j1 * W].rearrange(
                            "p (s i) -> p s i", i=W),
                        st["rcps"][grp][:, j0 * W:j1 * W].rearrange(
                            "p (s i) -> p s i", i=W),
                        mybir.AluOpType.mult,
                    )

                def emit_oproj(c_, chans=(0, 1)):
                    cs = c_ * P
                    for ch in chans:
                        y_ps = pp.tile([P, 512], f32, tag="pp", name="y_ps")
                        for ot in range(8):
                            nc.tensor.matmul(
                                y_ps, OT[:, ot, cs:cs + P],
                                woT[:, ot, ch * 512:(ch + 1) * 512],
                                start=(ot == 0), stop=(ot == 7),
                            )
                        y_sb = sp.tile([P, 512], bf16, tag="ysb", name="ysb")
                        nc.scalar.copy(y_sb[:], y_ps[:])
                        nc.sync.dma_start(
                            out_e.ap()[cs:cs + P, ch * 512:(ch + 1) * 512],
                            y_sb[:],
                        )

                # final O-proj chunk as two half-chains: slots 0-3 fire
                # after the last block's first normalize (fills the PE while
                # the last exp chain runs), slots 4-7 + eviction after the
                # second
                cs3 = (QC - 1) * P
                y3 = [None, None]

                def emit_y3(part, ots=None, final=False):
                    # part 0 consumes the even OT slots (written by norm grp
                    # 0 = kv-groups 0,1), part 1 the odd slots; accumulation
                    # order over slots is free
                    if ots is None:
                        ots = range(part, 8, 2)
                    for ch in range(2):
                        if part == 0:
                            y3[ch] = pp.tile([P, 512], f32, tag="pp", name="y3")
                        for ot in ots:
                            nc.tensor.matmul(
                                y3[ch], OT[:, ot, cs3:cs3 + P],
                                woT[:, ot, ch * 512:(ch + 1) * 512],
                                start=(ot == 0), stop=(ot == 7),
                            )
                        if final:
                            y_sb = sp.tile([P, 512], bf16, tag="ysb", name="ysb")
                            nc.vector.tensor_copy(y_sb[:], y3[ch][:])
                            nc.sync.dma_start(
                                out_e.ap()[cs3:cs3 + P, ch * 512:(ch + 1) * 512],
                                y_sb[:],
                            )

                QS = [(b, g) for b in range(NB) for g in range(4)]
                NS = len(QS)
                emit_vproj(0)
                emit_vproj(1)
                emit_scores(*QS[0])
                emit_expmult(*QS[0])
                emit_scores(*QS[1])
                emit_expmult(*QS[1])
                emit_scores(*QS[2])
                emit_expmult(*QS[2])
                for i in range(NS):
                    b_, _q = QS[i]
                    emit_avden(*QS[i])
                    if i + 3 < NS:
                        emit_scores(*QS[i + 3])
                        emit_expmult(*QS[i + 3])
                    j = i % 4
                    if j == 1:
                        if b_ + 1 < NB - 1:
                            emit_vproj(b_ + 2)
                        emit_norm(b_, 0)
                        if b_ == NB - 1:
                            emit_y3(0)
                    elif j == 3:
                        if b_ == NB - 1:
                            emit_norm(b_, 1, half=0)
                            emit_y3(1, ots=(1, 3))
                            emit_norm(b_, 1, half=1)
                            emit_y3(1, ots=(5, 7), final=True)
                        else:
                            emit_norm(b_, 1)
                            if b_ >= 1:
                                emit_oproj(b_ - 1)
    nc.compile()
    return nc


def _shard_inputs(x, wq, wk, wv, wo, q_norm_w, k_norm_w):
    u = (np.asarray(q_norm_w, np.float32) * np.asarray(k_norm_w, np.float32))

    def ktile(wT):  # [DIM, O] -> [128, 8, O] bf16 (k-tiled)
        return np.ascontiguousarray(
            wT.astype(BF16).reshape(8, P, -1).transpose(1, 0, 2))

    wqT = ktile(np.asarray(wq, np.float32).T)
    wkT = ktile(np.asarray(wk, np.float32).T)
    wvT = ktile(np.asarray(wv, np.float32).T)
    woT = ktile(np.asarray(wo, np.float32).T)  # wo[e, o] -> [o, e], contraction o
    uvec = np.tile(u, 2).reshape(P, 1).astype(np.float32)
    oblk2 = np.zeros((2, P), np.float32)
    oblk2[0, :64] = 1.0
    oblk2[1, 64:] = 1.0
    oblk2 = oblk2.astype(BF16)
    ebTr = _exp_bias_t(edge=False)

    in_maps = []
    for c in range(8):
        b, j = c // 4, c % 4
        xh = np.zeros((LX, DIM), np.float32)
        lo = j * LQ - HALO
        s0, s1 = max(lo, 0), min(j * LQ + NB * BQ, L)
        xh[s0 - lo:s1 - lo] = x[b, s0:s1]
        xtc = np.ascontiguousarray(
            xh.T.astype(BF16).reshape(8, P, LX).transpose(1, 0, 2))
        ebT0 = _exp_bias_t(edge=(j == 0))
        in_maps.append({
            "xt": xtc, "wqT": wqT, "wkT": wkT, "wvT": wvT, "woT": woT,
            "ebT0": ebT0, "ebTr": ebTr, "uvec": uvec, "oblk2": oblk2,
        })
    return in_maps


def _run(inputs, trace=False):
    global _compiled
    in_maps = _shard_inputs(**inputs)
    if _compiled is None:
        _compiled = _build()
    nc = _compiled
    res = run_bass_kernel_spmd(nc, in_maps, list(range(8)), trace=trace)
    full = np.empty((B, L, DIM), np.float32)
    for c in range(8):
        b, j = c // 4, c % 4
        full[b, j * LQ:(j + 1) * LQ] = res.results[c]["out"].astype(np.float32)
    return full, res


def kernel(x, wq, wk, wv, wo, q_norm_w, k_norm_w):
    full, _ = _run(dict(x=np.asarray(x), wq=np.asarray(wq), wk=np.asarray(wk),
                        wv=np.asarray(wv), wo=np.asarray(wo),
                        q_norm_w=np.asarray(q_norm_w),
                        k_norm_w=np.asarray(k_norm_w)))
    return full


# revision 61
# speedup vs baseline: 1.1109x; 1.1109x over previous
"""ALiBi sliding-window GQA attention on 8 Trainium2 NeuronCores.

Sharding: batch (2) x sequence quarter (4) -> 8 cores, each computing a
disjoint [512, 1024] output chunk from a 528-token input slice (16-token
halo on the left for the sliding window). No collectives needed.

Per-core kernel (bf16 compute, f32 accumulate), v3 dataflow:
  1. K projection runs weights-stationary so K^T materializes directly in
     [feature, token] layout (no PE transposes), accumulating per kt-tile
     as the xt DMA chunks stream in -- the PE starts ~5us earlier than a
     monolithic load allows.  K is NOT normalized in SBUF: its RMSNorm
     rstd (x0.125 score scale) is applied per-key-partition via the exp
     activation's scale vector, and q_norm_w*k_norm_w is folded into the
     PSUM->SBUF eviction scale.  Per-head sum-of-squares stats come from
     a block-diagonal ones matmul on the squared K^T.
  2. Q projection in [token, feature] layout (stats need free-axis
     reduction), RMSNorm'd, then transposed to [feature, token] via PE
     transpose-mode.
  3. 5 query blocks of 112 with 128-key windows; per kv-group scores come
     out PRE-TRANSPOSED: S.T[key, head-slot, query] = K^T.T @ Q^T.
       P.T = exp(rstd_k[key] * S.T) * ebT (host table: ALiBi slopes +
       causal/window mask as multiplicative zeros, pre-transposed,
       slot-permuted)
     row sums via ones-matmul, paired so one full-width reciprocal and
     one full-width normalize serve two kv-groups; normalization fused
     into the AV PSUM->SBUF eviction.  exp on Scalar, ebT-mult split
     GpSimd/Vector.
  4. Output projection uses OUT^T as the stationary operand so results
     land in [token, feature] layout for contiguous stores.
"""

import math

import numpy as np
import ml_dtypes

import concourse.bass as bass
import concourse.tile as tile
from concourse import bacc, mybir
from concourse.bass_utils import run_bass_kernel_spmd

BF16 = ml_dtypes.bfloat16

B, L, DIM = 2, 2048, 1024
N_HEADS, N_KV_HEADS, HEAD_DIM = 16, 4, 64
WINDOW = 16
EPS = 0.01

LQ = 512           # queries per core
HALO = WINDOW      # left halo
NB = 5             # query blocks per core (attention)
BQ = 112           # queries per block
BK = 128           # key window per block
LX = HALO + NB * BQ  # 576 = padded x slice width per core
LH = LX // 2       # 288: K-proj moving-operand half
P = 128
QC = 4             # Q/O projection chunks of 128 tokens (4*128 = LQ exactly)

# head h = g + 4j (g = kv group) -> scores slot 4g + j: each attention step
# batches the 4 heads of one kv group into a single matmul
SLOT = [4 * (h % 4) + h // 4 for h in range(N_HEADS)]

_compiled = None


def _alibi_slopes(n_heads):
    closest = 2 ** math.floor(math.log2(n_heads))
    base = 2.0 ** (-(2.0 ** (-(math.log2(closest) - 3))))
    slopes = base ** np.arange(1, closest + 1, dtype=np.float64)
    if closest < n_heads:
        eb = 2.0 ** (-(2.0 ** (-(math.log2(2 * closest) - 3))))
        extra = eb ** np.arange(1, 2 * (n_heads - closest) + 1, 2, dtype=np.float64)
        slopes = np.concatenate([slopes, extra])
    return slopes[:n_heads]


def _exp_bias_t(edge: bool) -> np.ndarray:
    """[BK, N_HEADS, BQ] transposed multiplicative softmax bias, slot order.

    Query i (block-local) sits at window column jk in [i, i+16]; entry is
    exp(slope_h * (jk - 16 - i)) inside the band, 0 outside.  With
    edge=True (first block of the sequence) keys at global position < 0
    (jk < 16) are additionally masked.
    """
    slopes = _alibi_slopes(N_HEADS)
    i = np.arange(BQ)[:, None]
    jk = np.arange(BK)[None, :]
    rel = jk - WINDOW - i                      # [BQ, BK]
    valid = (rel <= 0) & (rel >= -WINDOW)
    if edge:
        valid = valid & (jk >= WINDOW)
    arg = np.where(valid[None], slopes[:, None, None] * rel[None], -np.inf)
    eb = np.exp(arg)                           # [H, BQ, BK]
    perm = np.empty(N_HEADS, np.int64)
    perm[SLOT] = np.arange(N_HEADS)            # slot s holds head perm[s]
    return np.ascontiguousarray(eb[perm].transpose(2, 0, 1)).astype(BF16)


def _build():
    nc = bacc.Bacc("TRN2", target_bir_lowering=False, debug=False)
    f32, bf16 = mybir.dt.float32, mybir.dt.bfloat16

    xt_e = nc.dram_tensor("xt", [P, 8, LX], bf16, kind="ExternalInput")
    wq_e = nc.dram_tensor("wqT", [P, 8, 1024], bf16, kind="ExternalInput")
    wk_e = nc.dram_tensor("wkT", [P, 8, 256], bf16, kind="ExternalInput")
    wv_e = nc.dram_tensor("wvT", [P, 8, 256], bf16, kind="ExternalInput")
    wo_e = nc.dram_tensor("woT", [P, 8, 1024], bf16, kind="ExternalInput")
    eb0_e = nc.dram_tensor("ebT0", [BK, N_HEADS, BQ], bf16, kind="ExternalInput")
    ebr_e = nc.dram_tensor("ebTr", [BK, N_HEADS, BQ], bf16, kind="ExternalInput")
    u_e = nc.dram_tensor("uvec", [P, 1], f32, kind="ExternalInput")
    ob2_e = nc.dram_tensor("oblk2", [2, P], bf16, kind="ExternalInput")
    out_e = nc.dram_tensor("out", [LQ, DIM], bf16, kind="ExternalOutput")

    NQ = 2 * QC  # 8 Q-chunk tiles

    with tile.TileContext(nc) as tc:
        with (
            tc.tile_pool(name="w", bufs=1) as wp,
            tc.tile_pool(name="glob", bufs=1) as gp,
            tc.tile_pool(name="raw", bufs=NQ) as rp,
            tc.tile_pool(name="stage", bufs=4) as sp,
            tc.tile_pool(name="small", bufs=NQ) as mp,
            tc.tile_pool(name="att", bufs=3) as ap,
            tc.tile_pool(name="ptrs", bufs=2 * NB) as pt,
            tc.tile_pool(name="vpool", bufs=NB) as vp,
            tc.tile_pool(name="pp", bufs=2, space="PSUM") as pp,
        ):
            # ---- PE warmup: junk matmuls bridge the DMA-bound prologue so
            # the HAM clock-gate opens before real work arrives ----
            junk = wp.tile([P, 512], bf16)
            nc.vector.memset(junk[:], 1.0)
            ones64 = wp.tile([P, 64], bf16)
            nc.vector.memset(ones64[:], 1.0)
            epsv = wp.tile([P, 1], f32)
            nc.vector.memset(epsv[:], EPS)
            eps64v = wp.tile([P, 1], f32)
            nc.vector.memset(eps64v[:], HEAD_DIM * EPS)
            # block-diagonal ones: col j = 1 on partitions [64j, 64j+64)
            ones_blk = wp.tile([P, 2], bf16)
            nc.vector.memset(ones_blk[:], 0.0)
            nc.vector.memset(ones_blk[:64, 0:1], 1.0)
            nc.vector.memset(ones_blk[64:, 1:2], 1.0)
            wps = pp.tile([P, 512], f32, tag="pp", name="warm")[:64]
            for _ in range(10):
                nc.tensor.matmul(wps, ones64[:], junk[:], start=True, stop=True)

            # ---- input loads; wk first, then xt kt-pair by kt-pair so the
            # K projection starts accumulating as chunks land ----
            xt = wp.tile([P, 8, LX], bf16)
            wkT = wp.tile([P, 8, 256], bf16)
            wqT = wp.tile([P, 8, 1024], bf16)
            wvT = wp.tile([P, 8, 256], bf16)
            woT = wp.tile([P, 8, 1024], bf16)
            uvec = wp.tile([P, 1], f32)
            ones_blk2 = wp.tile([2, P], bf16)
            ebT0 = wp.tile([BK, N_HEADS, BQ], bf16)
            ebTr = wp.tile([BK, N_HEADS, BQ], bf16)
            nc.sync.dma_start(wkT[:], wk_e.ap())
            for g in range(4):
                nc.sync.dma_start(xt[:, 2 * g:2 * g + 2],
                                  xt_e.ap()[:, 2 * g:2 * g + 2])
            nc.sync.dma_start(uvec[:], u_e.ap())
            nc.sync.dma_start(ones_blk2[:], ob2_e.ap())
            for g in range(4):
                nc.sync.dma_start(wqT[:, 2 * g:2 * g + 2],
                                  wq_e.ap()[:, 2 * g:2 * g + 2])
            nc.sync.dma_start(wvT[:], wv_e.ap())
            nc.sync.dma_start(ebT0[:], eb0_e.ap())
            nc.sync.dma_start(ebTr[:], ebr_e.ap())
            for g in range(4):
                nc.sync.dma_start(woT[:, 2 * g:2 * g + 2],
                                  wo_e.ap()[:, 2 * g:2 * g + 2])

            QT = gp.tile([P, 8, LQ], bf16)    # normalized Q transposed
            KT = gp.tile([P, 2, LX], bf16)    # raw K^T, u-scaled
            sqKT = gp.tile([P, 2, LX], bf16)  # squared raw K^T (stats)
            OT = gp.tile([P, 8, LQ], bf16)    # attention out transposed

            vbs = []
            rks = []
            with (
                tc.tile_pool(name="kacc", bufs=4, space="PSUM") as ka,
                tc.tile_pool(name="kst", bufs=2, space="PSUM") as ks,
            ):
                # ---- phase 1a: K projection, weights-stationary ----
                kaccs = [ka.tile([P, 512], f32, tag="ka", name="kacc")[:, :LH]
                         for _ in range(4)]
                for kt in range(8):
                    for fh in range(2):
                        for lh in range(2):
                            nc.tensor.matmul(
                                kaccs[2 * fh + lh],
                                wkT[:, kt, fh * P:(fh + 1) * P],
                                xt[:, kt, lh * LH:(lh + 1) * LH],
                                start=(kt == 0), stop=(kt == 7),
                            )
                for fh in range(2):
                    for lh in range(2):
                        acc = kaccs[2 * fh + lh]
                        nc.scalar.activation(
                            KT[:, fh, lh * LH:(lh + 1) * LH], acc[:],
                            mybir.ActivationFunctionType.Copy, scale=uvec[:],
                        )
                        nc.scalar.square(
                            sqKT[:, fh, lh * LH:(lh + 1) * LH], acc[:])

                # ---- phase 1c: K RMSNorm stats per key window ----
                # rk = 0.125 / sqrt(mean(k^2) + eps) = 1/sqrt(sumsq + 64*eps)
                # per (window key partition, kv-head); consumed as the exp
                # activation's per-partition scale vector.
                for b_ in range(NB):
                    qs = b_ * BQ
                    kss = ks.tile([P, 4], f32, tag="ks", name="kss")
                    for fh in range(2):
                        nc.tensor.matmul(
                            kss[:, 2 * fh:2 * fh + 2],
                            sqKT[:, fh, qs:qs + BK],
                            ones_blk[:],
                            start=True, stop=True,
                        )
                    srtk = mp.tile([P, 8], f32, tag="srt", name="srtk")[:, :4]
                    nc.scalar.activation(
                        srtk[:], kss[:], mybir.ActivationFunctionType.Sqrt,
                        scale=1.0, bias=eps64v[:],
                    )
                    rk = mp.tile([P, 4], f32, tag="rk", name="rk", bufs=NB)
                    nc.vector.reciprocal(rk[:], srtk[:])
                    rks.append(rk)

            # ---- phase 2: Q projection, weights-stationary -> QT slot fc
            # directly (fc = 128-feature chunk = QT slot = heads 2fc,2fc+1).
            # Per-head stats via the same block-diagonal-ones matmul as K;
            # rstd replicated across each head's 64 partitions by a tiny
            # contraction-2 matmul, then one Vector multiply lands the
            # normalized QT slot.  No PE transposes, no free-axis reduces.
            qraws, sqs, rstds = [], [], []
            with (
                tc.tile_pool(name="qst", bufs=2, space="PSUM") as qsp,
                tc.tile_pool(name="qrp", bufs=2, space="PSUM") as qrp,
            ):
                def emit_qchain(fc):
                    q_ps = pp.tile([P, 512], f32, tag="pp", name="q_ps")
                    for kt in range(8):
                        nc.tensor.matmul(
                            q_ps,
                            wqT[:, kt, fc * P:(fc + 1) * P],
                            xt[:, kt, HALO:HALO + LQ],
                            start=(kt == 0), stop=(kt == 7),
                        )
                    qraw = rp.tile([P, 512], bf16, tag="raw", name="qraw")
                    nc.scalar.copy(qraw[:], q_ps[:])
                    sq = sp.tile([P, 512], bf16, tag="sq", name="sq")
                    nc.scalar.square(sq[:], q_ps[:])
                    qraws.append(qraw)
                    sqs.append(sq)

                def emit_qstats(fc):
                    qst = qsp.tile([2, 512], f32, tag="qst", name="qst")
                    nc.tensor.matmul(qst[:], ones_blk[:], sqs[fc][:],
                                     start=True, stop=True)
                    srt = mp.tile([2, 512], f32, tag="qsrt", name="qsrt",
                                  bufs=2)
                    nc.scalar.activation(
                        srt[:], qst[:], mybir.ActivationFunctionType.Sqrt,
                        scale=1.0 / HEAD_DIM, bias=epsv[:2],
                    )
                    rstd_f = mp.tile([2, 512], f32, tag="qrstdf", name="qrstdf",
                                     bufs=2)
                    nc.vector.reciprocal_approx_fast(rstd_f[:], srt[:])
                    rstd = mp.tile([2, 512], bf16, tag="qrstd", name="qrstd",
                                   bufs=2)
                    nc.vector.tensor_copy(rstd[:], rstd_f[:])
                    rstds.append(rstd)

                def emit_qrep(fc):
                    rep = qrp.tile([P, 512], f32, tag="qrep", name="qrep")
                    nc.tensor.matmul(rep[:], ones_blk2[:], rstds[fc][:],
                                     start=True, stop=True)
                    nc.vector.tensor_tensor(
                        QT[:, fc, :], rep[:], qraws[fc][:],
                        mybir.AluOpType.mult,
                    )

                for fc in range(8):
                    emit_qchain(fc)
                    if fc >= 1:
                        emit_qstats(fc - 1)
                    if fc >= 2:
                        emit_qrep(fc - 2)
                emit_qstats(7)
                emit_qrep(6)
                emit_qrep(7)

                # dummy exp: forces the Exp<->Sqrt ACT-table swap to happen
                # here instead of on the first real exp's critical path
                dume = mp.tile([P, 8], f32, tag="dume", name="dume")
                nc.scalar.activation(
                    dume[:, :1], epsv[:], mybir.ActivationFunctionType.Exp)

            with (
                tc.tile_pool(name="pot", bufs=2, space="PSUM") as pot,
                tc.tile_pool(name="psc", bufs=4, space="PSUM") as psc,
            ):
                # ---- phase 4: attention + output projection ----
                # Flat software-pipelined stream over NB*4 kv-group steps.
                # All 4 heads of kv-group g share the same K/V stationary
                # operand, so each stage is ONE matmul with the 4 heads
                # batched in the moving operand (448 cols).  Head h = g + 4j
                # lives at score-slot (g, j); packed flat [4W] in PSUM.  The
                # PE queue runs scores(i+2) between scores(i) and AV(i) so
                # the exp -> ebT-mult chain latency is hidden by real matmul
                # work even across block boundaries; O-projection chunks
                # slot in at block boundaries as extra filler.
                blocks = []
                for b_ in range(NB):
                    blocks.append(dict(
                        qs=b_ * BQ,
                        W=BQ if b_ < NB - 1 else LQ - (NB - 1) * BQ,
                        ebT=ebT0 if b_ == 0 else ebTr,
                        rk=rks[b_],
                        ot_g={}, rcps={}, dens={}, scs={}, ptrs={},
                    ))

                def emit_vproj(b_):
                    # V projection for block b_'s key window, deferred into
                    # the attention stream as PE filler (vb is first needed
                    # at block b_'s AV step)
                    qs = b_ * BQ
                    v_ps = pp.tile([P, 512], f32, tag="pp", name="v_ps")[:, :256]
                    for kt in range(8):
                        nc.tensor.matmul(
                            v_ps, xt[:, kt, qs:qs + BK], wvT[:, kt],
                            start=(kt == 0), stop=(kt == 7),
                        )
                    vb = vp.tile([P, 256], bf16, tag="vb", name="vb")
                    nc.scalar.copy(vb[:], v_ps[:])
                    vbs.append(vb)

                def emit_scores(b_, g):
                    st = blocks[b_]
                    qs, W = st["qs"], st["W"]
                    sc = psc.tile([P, 4 * BQ], f32, tag="sc", name="sc")
                    # moving: the 4 heads {g+4j} = OT-slots g//2 + 2j of QT
                    mv = QT[:].rearrange("p (j s) l -> p s j l", s=2)[
                        (g % 2) * 64:(g % 2) * 64 + 64, g // 2, :, qs:qs + W]
                    nc.tensor.matmul(
                        sc[:, :4 * W],
                        KT[(g % 2) * 64:(g % 2) * 64 + 64, g // 2, qs:qs + BK],
                        mv,
                        start=True, stop=True,
                    )
                    st["scs"][g] = sc

                def emit_expmult(b_, g):
                    st = blocks[b_]
                    W = st["W"]
                    e_t = ap.tile([P, 4 * BQ], bf16, tag="et", name="e_t")
                    nc.scalar.activation(
                        e_t[:, :4 * W], st["scs"][g][:, :4 * W],
                        mybir.ActivationFunctionType.Exp,
                        scale=st["rk"][:, g:g + 1],
                    )
                    # mask+ALiBi multiply split across GpSimd (slots 0-1)
                    # and Vector (slots 2-3)
                    ptr = pt.tile([P, 4 * BQ], bf16, tag="ptr", name="ptr")
                    nc.gpsimd.tensor_tensor(
                        ptr[:, :2 * W].rearrange("p (s i) -> p s i", i=W),
                        e_t[:, :2 * W].rearrange("p (s i) -> p s i", i=W),
                        st["ebT"][:, 4 * g:4 * g + 2, :W],
                        mybir.AluOpType.mult,
                    )
                    nc.vector.tensor_tensor(
                        ptr[:, 2 * W:4 * W].rearrange("p (s i) -> p s i", i=W),
                        e_t[:, 2 * W:4 * W].rearrange("p (s i) -> p s i", i=W),
                        st["ebT"][:, 4 * g + 2:4 * g + 4, :W],
                        mybir.AluOpType.mult,
                    )
                    st["ptrs"][g] = ptr

                def emit_avden(b_, g):
                    # den for the kv-group pair (grp = g//2) lands in one
                    # [128, 4W] tile -- g even in rows 0-63, g odd in
                    # 64-127 -- so ONE full-width reciprocal and ONE
                    # full-width normalize serve both groups (DVE time
                    # scales with elems/partition, not partitions).
                    st = blocks[b_]
                    W = st["W"]
                    grp = g // 2
                    if grp not in st["ot_g"]:
                        st["ot_g"][grp] = pot.tile([P, 4 * BQ], f32, tag="ot",
                                                   name="ot_g")
                    ptr = st["ptrs"][g]
                    hb = (g % 2) * 64
                    nc.tensor.matmul(
                        st["ot_g"][grp][hb:hb + 64, :4 * W],
                        vbs[b_][:, g * 64:(g + 1) * 64],
                        ptr[:, :4 * W],
                        start=True, stop=True,
                    )
                    if g % 2 == 0:
                        st["dens"][grp] = psc.tile([P, 4 * BQ], f32, tag="sc",
                                                   name="den")
                    den = st["dens"][grp]
                    nc.tensor.matmul(
                        den[hb:hb + 64, :4 * W], ones64[:], ptr[:, :4 * W],
                        start=True, stop=True,
                    )
                    if g % 2 == 1:
                        rcp = sp.tile([P, 4 * BQ], f32, tag="rcp", name="rcp")
                        nc.vector.reciprocal_approx_fast(
                            rcp[:, :4 * W], den[:, :4 * W])
                        st["rcps"][grp] = rcp

                def emit_norm(b_, grp, half=None):
                    # grp 0 = kv-groups 0,1 (OT slots 0,2,4,6); grp 1 = 2,3.
                    # half=0/1 emits only the lower/upper two head-slots (the
                    # last block splits so the final O-proj chain can start
                    # after the first half).
                    st = blocks[b_]
                    qs, W = st["qs"], st["W"]
                    j0, j1 = (0, 4) if half is None else (2 * half, 2 * half + 2)
                    nc.vector.tensor_tensor(
                        OT[:].rearrange("p (j s) l -> p s j l", s=2)[
                            :, grp, j0:j1, qs:qs + W],
                        st["ot_g"][grp][:, j0 * W:j1 * W].rearrange(
                            "p (s i) -> p s i", i=W),
                        st["rcps"][grp][:, j0 * W:j1 * W].rearrange(
                            "p (s i) -> p s i", i=W),
                        mybir.AluOpType.mult,
                    )

                def emit_oproj(c_, chans=(0, 1)):
                    cs = c_ * P
                    for ch in chans:
                        y_ps = pp.tile([P, 512], f32, tag="pp", name="y_ps")
                        for ot in range(8):
                            nc.tensor.matmul(
                                y_ps, OT[:, ot, cs:cs + P],
                                woT[:, ot, ch * 512:(ch + 1) * 512],
                                start=(ot == 0), stop=(ot == 7),
                            )
                        y_sb = sp.tile([P, 512], bf16, tag="ysb", name="ysb")
                        nc.scalar.copy(y_sb[:], y_ps[:])
                        nc.sync.dma_start(
                            out_e.ap()[cs:cs + P, ch * 512:(ch + 1) * 512],
                            y_sb[:],
                        )

                # final O-proj chunk as two half-chains: slots 0-3 fire
                # after the last block's first normalize (fills the PE while
                # the last exp chain runs), slots 4-7 + eviction after the
                # second
                cs3 = (QC - 1) * P
                y3 = [None, None]

                def emit_y3(part, ots=None, final=False):
                    # part 0 consumes the even OT slots (written by norm grp
                    # 0 = kv-groups 0,1), part 1 the odd slots; accumulation
                    # order over slots is free
                    if ots is None:
                        ots = range(part, 8, 2)
                    for ch in range(2):
                        if part == 0:
                            y3[ch] = pp.tile([P, 512], f32, tag="pp", name="y3")
                        for ot in ots:
                            nc.tensor.matmul(
                                y3[ch], OT[:, ot, cs3:cs3 + P],
                                woT[:, ot, ch * 512:(ch + 1) * 512],
                                start=(ot == 0), stop=(ot == 7),
                            )
                        if final:
                            y_sb = sp.tile([P, 512], bf16, tag="ysb", name="ysb")
                            nc.vector.tensor_copy(y_sb[:], y3[ch][:])
                            nc.sync.dma_start(
                                out_e.ap()[cs3:cs3 + P, ch * 512:(ch + 1) * 512],
                                y_sb[:],
                            )

                QS = [(b, g) for b in range(NB) for g in range(4)]
                NS = len(QS)
                emit_vproj(0)
                emit_vproj(1)
                emit_scores(*QS[0])
                emit_expmult(*QS[0])
                emit_scores(*QS[1])
                emit_expmult(*QS[1])
                emit_scores(*QS[2])
                emit_expmult(*QS[2])
                for i in range(NS):
                    b_, _q = QS[i]
                    emit_avden(*QS[i])
                    if i + 3 < NS:
                        emit_scores(*QS[i + 3])
                        emit_expmult(*QS[i + 3])
                    j = i % 4
                    if j == 1:
                        if b_ + 1 < NB - 1:
                            emit_vproj(b_ + 2)
                        emit_norm(b_, 0)
                        if b_ == NB - 1:
                            emit_y3(0)
                    elif j == 3:
                        if b_ == NB - 1:
                            emit_norm(b_, 1, half=0)
                            emit_y3(1, ots=(1, 3))
                            emit_norm(b_, 1, half=1)
                            emit_y3(1, ots=(5, 7), final=True)
                        else:
                            emit_norm(b_, 1)
                            if b_ >= 1:
                                emit_oproj(b_ - 1)
    nc.compile()
    return nc


def _shard_inputs(x, wq, wk, wv, wo, q_norm_w, k_norm_w):
    u = (np.asarray(q_norm_w, np.float32) * np.asarray(k_norm_w, np.float32))

    def ktile(wT):  # [DIM, O] -> [128, 8, O] bf16 (k-tiled)
        return np.ascontiguousarray(
            wT.astype(BF16).reshape(8, P, -1).transpose(1, 0, 2))

    wqT = ktile(np.asarray(wq, np.float32).T)
    wkT = ktile(np.asarray(wk, np.float32).T)
    wvT = ktile(np.asarray(wv, np.float32).T)
    woT = ktile(np.asarray(wo, np.float32).T)  # wo[e, o] -> [o, e], contraction o
    uvec = np.tile(u, 2).reshape(P, 1).astype(np.float32)
    oblk2 = np.zeros((2, P), np.float32)
    oblk2[0, :64] = 1.0
    oblk2[1, 64:] = 1.0
    oblk2 = oblk2.astype(BF16)
    ebTr = _exp_bias_t(edge=False)

    in_maps = []
    for c in range(8):
        b, j = c // 4, c % 4
        xh = np.zeros((LX, DIM), np.float32)
        lo = j * LQ - HALO
        s0, s1 = max(lo, 0), min(j * LQ + NB * BQ, L)
        xh[s0 - lo:s1 - lo] = x[b, s0:s1]
        xtc = np.ascontiguousarray(
            xh.T.astype(BF16).reshape(8, P, LX).transpose(1, 0, 2))
        ebT0 = _exp_bias_t(edge=(j == 0))
        in_maps.append({
            "xt": xtc, "wqT": wqT, "wkT": wkT, "wvT": wvT, "woT": woT,
            "ebT0": ebT0, "ebTr": ebTr, "uvec": uvec, "oblk2": oblk2,
        })
    return in_maps


def _run(inputs, trace=False):
    global _compiled
    in_maps = _shard_inputs(**inputs)
    if _compiled is None:
        _compiled = _build()
    nc = _compiled
    res = run_bass_kernel_spmd(nc, in_maps, list(range(8)), trace=trace)
    full = np.empty((B, L, DIM), np.float32)
    for c in range(8):
        b, j = c // 4, c % 4
        full[b, j * LQ:(j + 1) * LQ] = res.results[c]["out"].astype(np.float32)
    return full, res


def kernel(x, wq, wk, wv, wo, q_norm_w, k_norm_w):
    full, _ = _run(dict(x=np.asarray(x), wq=np.asarray(wq), wk=np.asarray(wk),
                        wv=np.asarray(wv), wo=np.asarray(wo),
                        q_norm_w=np.asarray(q_norm_w),
                        k_norm_w=np.asarray(k_norm_w)))
    return full


# revision 62
# speedup vs baseline: 1.2196x; 1.0979x over previous
"""ALiBi sliding-window GQA attention on 8 Trainium2 NeuronCores.

Sharding: batch (2) x sequence quarter (4) -> 8 cores, each computing a
disjoint [512, 1024] output chunk from a 528-token input slice (16-token
halo on the left for the sliding window). No collectives needed.

Per-core kernel (bf16 compute, f32 accumulate), v3 dataflow:
  1. K projection runs weights-stationary so K^T materializes directly in
     [feature, token] layout (no PE transposes), accumulating per kt-tile
     as the xt DMA chunks stream in -- the PE starts ~5us earlier than a
     monolithic load allows.  K is NOT normalized in SBUF: its RMSNorm
     rstd (x0.125 score scale) is applied per-key-partition via the exp
     activation's scale vector, and q_norm_w*k_norm_w is folded into the
     PSUM->SBUF eviction scale.  Per-head sum-of-squares stats come from
     a block-diagonal ones matmul on the squared K^T.
  2. Q projection in [token, feature] layout (stats need free-axis
     reduction), RMSNorm'd, then transposed to [feature, token] via PE
     transpose-mode.
  3. 5 query blocks of 112 with 128-key windows; per kv-group scores come
     out PRE-TRANSPOSED: S.T[key, head-slot, query] = K^T.T @ Q^T.
       P.T = exp(rstd_k[key] * S.T) * ebT (host table: ALiBi slopes +
       causal/window mask as multiplicative zeros, pre-transposed,
       slot-permuted)
     row sums via ones-matmul, paired so one full-width reciprocal and
     one full-width normalize serve two kv-groups; normalization fused
     into the AV PSUM->SBUF eviction.  exp on Scalar, ebT-mult split
     GpSimd/Vector.
  4. Output projection uses OUT^T as the stationary operand so results
     land in [token, feature] layout for contiguous stores.
"""

import math

import numpy as np
import ml_dtypes

import concourse.bass as bass
import concourse.tile as tile
from concourse import bacc, mybir
from concourse.bass_utils import run_bass_kernel_spmd

BF16 = ml_dtypes.bfloat16

B, L, DIM = 2, 2048, 1024
N_HEADS, N_KV_HEADS, HEAD_DIM = 16, 4, 64
WINDOW = 16
EPS = 0.01

LQ = 512           # queries per core
HALO = WINDOW      # left halo
NB = 5             # query blocks per core (attention)
BQ = 112           # queries per block
BK = 128           # key window per block
LX = HALO + NB * BQ  # 576 = padded x slice width per core
LH = LX // 2       # 288: K-proj moving-operand half
P = 128
QC = 4             # Q/O projection chunks of 128 tokens (4*128 = LQ exactly)

# head h = g + 4j (g = kv group) -> scores slot 4g + j: each attention step
# batches the 4 heads of one kv group into a single matmul
SLOT = [4 * (h % 4) + h // 4 for h in range(N_HEADS)]

_compiled = None


def _alibi_slopes(n_heads):
    closest = 2 ** math.floor(math.log2(n_heads))
    base = 2.0 ** (-(2.0 ** (-(math.log2(closest) - 3))))
    slopes = base ** np.arange(1, closest + 1, dtype=np.float64)
    if closest < n_heads:
        eb = 2.0 ** (-(2.0 ** (-(math.log2(2 * closest) - 3))))
        extra = eb ** np.arange(1, 2 * (n_heads - closest) + 1, 2, dtype=np.float64)
        slopes = np.concatenate([slopes, extra])
    return slopes[:n_heads]


def _exp_bias_t(edge: bool) -> np.ndarray:
    """[BK, N_HEADS, BQ] transposed multiplicative softmax bias, slot order.

    Query i (block-local) sits at window column jk in [i, i+16]; entry is
    exp(slope_h * (jk - 16 - i)) inside the band, 0 outside.  With
    edge=True (first block of the sequence) keys at global position < 0
    (jk < 16) are additionally masked.
    """
    slopes = _alibi_slopes(N_HEADS)
    i = np.arange(BQ)[:, None]
    jk = np.arange(BK)[None, :]
    rel = jk - WINDOW - i                      # [BQ, BK]
    valid = (rel <= 0) & (rel >= -WINDOW)
    if edge:
        valid = valid & (jk >= WINDOW)
    arg = np.where(valid[None], slopes[:, None, None] * rel[None], -np.inf)
    eb = np.exp(arg)                           # [H, BQ, BK]
    perm = np.empty(N_HEADS, np.int64)
    perm[SLOT] = np.arange(N_HEADS)            # slot s holds head perm[s]
    return np.ascontiguousarray(eb[perm].transpose(2, 0, 1)).astype(BF16)


def _build():
    nc = bacc.Bacc("TRN2", target_bir_lowering=False, debug=False)
    f32, bf16 = mybir.dt.float32, mybir.dt.bfloat16

    xt_e = nc.dram_tensor("xt", [P, 8, LX], bf16, kind="ExternalInput")
    wq_e = nc.dram_tensor("wqT", [P, 8, 1024], bf16, kind="ExternalInput")
    wk_e = nc.dram_tensor("wkT", [P, 8, 256], bf16, kind="ExternalInput")
    wv_e = nc.dram_tensor("wvT", [P, 8, 256], bf16, kind="ExternalInput")
    wo_e = nc.dram_tensor("woT", [P, 8, 1024], bf16, kind="ExternalInput")
    eb0_e = nc.dram_tensor("ebT0", [BK, N_HEADS, BQ], bf16, kind="ExternalInput")
    ebr_e = nc.dram_tensor("ebTr", [BK, N_HEADS, BQ], bf16, kind="ExternalInput")
    id_e = nc.dram_tensor("ident", [P, P], bf16, kind="ExternalInput")
    u_e = nc.dram_tensor("uvec", [P, 1], f32, kind="ExternalInput")
    out_e = nc.dram_tensor("out", [LQ, DIM], bf16, kind="ExternalOutput")

    NQ = 2 * QC  # 8 Q-chunk tiles

    with tile.TileContext(nc) as tc:
        with (
            tc.tile_pool(name="w", bufs=1) as wp,
            tc.tile_pool(name="glob", bufs=1) as gp,
            tc.tile_pool(name="raw", bufs=NQ) as rp,
            tc.tile_pool(name="stage", bufs=4) as sp,
            tc.tile_pool(name="small", bufs=NQ) as mp,
            tc.tile_pool(name="att", bufs=3) as ap,
            tc.tile_pool(name="ptrs", bufs=2 * NB) as pt,
            tc.tile_pool(name="vpool", bufs=NB) as vp,
            tc.tile_pool(name="pp", bufs=2, space="PSUM") as pp,
        ):
            # ---- PE warmup: junk matmuls bridge the DMA-bound prologue so
            # the HAM clock-gate opens before real work arrives ----
            junk = wp.tile([P, 512], bf16)
            nc.vector.memset(junk[:], 1.0)
            ones64 = wp.tile([P, 64], bf16)
            nc.vector.memset(ones64[:], 1.0)
            epsv = wp.tile([P, 1], f32)
            nc.vector.memset(epsv[:], EPS)
            eps64v = wp.tile([P, 1], f32)
            nc.vector.memset(eps64v[:], HEAD_DIM * EPS)
            # block-diagonal ones: col j = 1 on partitions [64j, 64j+64)
            ones_blk = wp.tile([P, 2], bf16)
            nc.vector.memset(ones_blk[:], 0.0)
            nc.vector.memset(ones_blk[:64, 0:1], 1.0)
            nc.vector.memset(ones_blk[64:, 1:2], 1.0)
            wps = pp.tile([P, 512], f32, tag="pp", name="warm")[:64]
            for _ in range(10):
                nc.tensor.matmul(wps, ones64[:], junk[:], start=True, stop=True)

            # ---- input loads; wk first, then xt kt-pair by kt-pair so the
            # K projection starts accumulating as chunks land ----
            xt = wp.tile([P, 8, LX], bf16)
            wkT = wp.tile([P, 8, 256], bf16)
            wqT = wp.tile([P, 8, 1024], bf16)
            wvT = wp.tile([P, 8, 256], bf16)
            woT = wp.tile([P, 8, 1024], bf16)
            uvec = wp.tile([P, 1], f32)
            ebT0 = wp.tile([BK, N_HEADS, BQ], bf16)
            ebTr = wp.tile([BK, N_HEADS, BQ], bf16)
            ident = wp.tile([P, P], bf16)
            nc.sync.dma_start(wkT[:], wk_e.ap())
            for g in range(4):
                nc.sync.dma_start(xt[:, 2 * g:2 * g + 2],
                                  xt_e.ap()[:, 2 * g:2 * g + 2])
            nc.sync.dma_start(uvec[:], u_e.ap())
            for g in range(4):
                nc.sync.dma_start(wqT[:, 2 * g:2 * g + 2],
                                  wq_e.ap()[:, 2 * g:2 * g + 2])
            nc.sync.dma_start(wvT[:], wv_e.ap())
            nc.sync.dma_start(ebT0[:], eb0_e.ap())
            nc.sync.dma_start(ebTr[:], ebr_e.ap())
            nc.sync.dma_start(ident[:], id_e.ap())
            for g in range(4):
                nc.sync.dma_start(woT[:, 2 * g:2 * g + 2],
                                  wo_e.ap()[:, 2 * g:2 * g + 2])

            QT = gp.tile([P, 8, LQ], bf16)    # normalized Q transposed
            KT = gp.tile([P, 2, LX], bf16)    # raw K^T, u-scaled
            sqKT = gp.tile([P, 2, LX], bf16)  # squared raw K^T (stats)
            OT = gp.tile([P, 8, LQ], bf16)    # attention out transposed

            vbs = []
            rks = []
            with (
                tc.tile_pool(name="kacc", bufs=4, space="PSUM") as ka,
                tc.tile_pool(name="kst", bufs=2, space="PSUM") as ks,
            ):
                # ---- phase 1a: K projection, weights-stationary ----
                kaccs = [ka.tile([P, 512], f32, tag="ka", name="kacc")[:, :LH]
                         for _ in range(4)]
                for kt in range(8):
                    for fh in range(2):
                        for lh in range(2):
                            nc.tensor.matmul(
                                kaccs[2 * fh + lh],
                                wkT[:, kt, fh * P:(fh + 1) * P],
                                xt[:, kt, lh * LH:(lh + 1) * LH],
                                start=(kt == 0), stop=(kt == 7),
                            )
                for fh in range(2):
                    for lh in range(2):
                        acc = kaccs[2 * fh + lh]
                        nc.scalar.activation(
                            KT[:, fh, lh * LH:(lh + 1) * LH], acc[:],
                            mybir.ActivationFunctionType.Copy, scale=uvec[:],
                        )
                        nc.scalar.square(
                            sqKT[:, fh, lh * LH:(lh + 1) * LH], acc[:])

                # ---- phase 1c: K RMSNorm stats per key window ----
                # rk = 0.125 / sqrt(mean(k^2) + eps) = 1/sqrt(sumsq + 64*eps)
                # per (window key partition, kv-head); consumed as the exp
                # activation's per-partition scale vector.
                for b_ in range(NB):
                    qs = b_ * BQ
                    kss = ks.tile([P, 4], f32, tag="ks", name="kss")
                    for fh in range(2):
                        nc.tensor.matmul(
                            kss[:, 2 * fh:2 * fh + 2],
                            sqKT[:, fh, qs:qs + BK],
                            ones_blk[:],
                            start=True, stop=True,
                        )
                    srtk = mp.tile([P, 8], f32, tag="srt", name="srtk")[:, :4]
                    nc.scalar.activation(
                        srtk[:], kss[:], mybir.ActivationFunctionType.Sqrt,
                        scale=1.0, bias=eps64v[:],
                    )
                    rk = mp.tile([P, 4], f32, tag="rk", name="rk", bufs=NB)
                    nc.vector.reciprocal(rk[:], srtk[:])
                    rks.append(rk)

            with (
                tc.tile_pool(name="pot", bufs=2, space="PSUM") as pot,
                tc.tile_pool(name="psc", bufs=4, space="PSUM") as psc,
            ):
                # ---- phase 2: Q projection -> raw [token, feature] ----
                raws = []
                for c_ in range(QC):
                    cs = c_ * P
                    for ch in range(2):
                        q_ps = pp.tile([P, 512], f32, tag="pp", name="q_ps")
                        for kt in range(8):
                            nc.tensor.matmul(
                                q_ps,
                                xt[:, kt, HALO + cs:HALO + cs + P],
                                wqT[:, kt, ch * 512:(ch + 1) * 512],
                                start=(kt == 0), stop=(kt == 7),
                            )
                        raw = rp.tile([P, 512], bf16, tag="raw", name="q_raw")
                        nc.scalar.copy(raw[:], q_ps[:])
                        raws.append(raw)

                # ---- Q RMSNorm, per-chain so each hat completes ~2.5us
                # after its projection chain (Copy/Square/Sqrt share one
                # ACT table set, so no LUT thrash) ----
                hats = []
                for raw in raws:
                    sq = sp.tile([P, 512], bf16, tag="sq", name="sq")
                    nc.scalar.square(sq[:], raw[:])
                    ss = mp.tile([P, 8], f32, tag="ss", name="ss")
                    nc.vector.reduce_sum(
                        ss[:], sq[:].rearrange("l (h d) -> l h d", d=HEAD_DIM),
                        axis=mybir.AxisListType.X,
                    )
                    srt = mp.tile([P, 8], f32, tag="srt", name="srt")
                    nc.scalar.activation(
                        srt[:], ss[:], mybir.ActivationFunctionType.Sqrt,
                        scale=1.0 / HEAD_DIM, bias=epsv[:],
                    )
                    rstd = mp.tile([P, 8], f32, tag="rstd", name="rstd")
                    nc.vector.reciprocal(rstd[:], srt[:])
                    hat = rp.tile([P, 512], bf16, tag="hat", name="hat")
                    nc.vector.tensor_tensor(
                        hat[:].rearrange("l (h d) -> l h d", d=HEAD_DIM),
                        raw[:].rearrange("l (h d) -> l h d", d=HEAD_DIM),
                        rstd[:, :, None].to_broadcast((P, 8, HEAD_DIM)),
                        mybir.AluOpType.mult,
                    )
                    hats.append(hat)

                # dummy exp: forces the Exp<->Sqrt ACT-table swap to happen
                # here (during the transpose phase) instead of on the first
                # real exp's critical path
                dume = mp.tile([P, 8], f32, tag="srt", name="dume")
                nc.scalar.activation(
                    dume[:, :1], epsv[:], mybir.ActivationFunctionType.Exp)

                def emit_qtrans(c_, ch):
                    cs = c_ * P
                    hat = hats[2 * c_ + ch]
                    tp = pp.tile([P, 4, P], bf16, tag="pp", name="tpq")
                    for ot in range(4):
                        nc.tensor.transpose(
                            tp[:, ot], hat[:, ot * P:(ot + 1) * P], ident[:])
                    dst = QT[:, ch * 4:ch * 4 + 4, cs:cs + P]
                    if ch % 2 == 0:
                        nc.vector.tensor_copy(dst, tp[:])
                    else:
                        nc.scalar.copy(dst, tp[:])

                # ---- phase 4: attention + output projection ----
                # Flat software-pipelined stream over NB*4 kv-group steps.
                # All 4 heads of kv-group g share the same K/V stationary
                # operand, so each stage is ONE matmul with the 4 heads
                # batched in the moving operand (448 cols).  Head h = g + 4j
                # lives at score-slot (g, j); packed flat [4W] in PSUM.  The
                # PE queue runs scores(i+2) between scores(i) and AV(i) so
                # the exp -> ebT-mult chain latency is hidden by real matmul
                # work even across block boundaries; O-projection chunks
                # slot in at block boundaries as extra filler.
                blocks = []
                for b_ in range(NB):
                    blocks.append(dict(
                        qs=b_ * BQ,
                        W=BQ if b_ < NB - 1 else LQ - (NB - 1) * BQ,
                        ebT=ebT0 if b_ == 0 else ebTr,
                        rk=rks[b_],
                        ot_g={}, rcps={}, dens={}, scs={}, ptrs={},
                    ))

                def emit_vproj(b_):
                    # V projection for block b_'s key window, deferred into
                    # the attention stream as PE filler (vb is first needed
                    # at block b_'s AV step)
                    qs = b_ * BQ
                    v_ps = pp.tile([P, 512], f32, tag="pp", name="v_ps")[:, :256]
                    for kt in range(8):
                        nc.tensor.matmul(
                            v_ps, xt[:, kt, qs:qs + BK], wvT[:, kt],
                            start=(kt == 0), stop=(kt == 7),
                        )
                    vb = vp.tile([P, 256], bf16, tag="vb", name="vb")
                    nc.scalar.copy(vb[:], v_ps[:])
                    vbs.append(vb)

                def emit_scores(b_, g):
                    st = blocks[b_]
                    qs, W = st["qs"], st["W"]
                    sc = psc.tile([P, 4 * BQ], f32, tag="sc", name="sc")
                    # moving: the 4 heads {g+4j} = OT-slots g//2 + 2j of QT
                    mv = QT[:].rearrange("p (j s) l -> p s j l", s=2)[
                        (g % 2) * 64:(g % 2) * 64 + 64, g // 2, :, qs:qs + W]
                    nc.tensor.matmul(
                        sc[:, :4 * W],
                        KT[(g % 2) * 64:(g % 2) * 64 + 64, g // 2, qs:qs + BK],
                        mv,
                        start=True, stop=True,
                    )
                    st["scs"][g] = sc

                def emit_expmult(b_, g):
                    st = blocks[b_]
                    W = st["W"]
                    e_t = ap.tile([P, 4 * BQ], bf16, tag="et", name="e_t")
                    nc.scalar.activation(
                        e_t[:, :4 * W], st["scs"][g][:, :4 * W],
                        mybir.ActivationFunctionType.Exp,
                        scale=st["rk"][:, g:g + 1],
                    )
                    # mask+ALiBi multiply split across GpSimd (slots 0-1)
                    # and Vector (slots 2-3)
                    ptr = pt.tile([P, 4 * BQ], bf16, tag="ptr", name="ptr")
                    nc.gpsimd.tensor_tensor(
                        ptr[:, :2 * W].rearrange("p (s i) -> p s i", i=W),
                        e_t[:, :2 * W].rearrange("p (s i) -> p s i", i=W),
                        st["ebT"][:, 4 * g:4 * g + 2, :W],
                        mybir.AluOpType.mult,
                    )
                    nc.vector.tensor_tensor(
                        ptr[:, 2 * W:4 * W].rearrange("p (s i) -> p s i", i=W),
                        e_t[:, 2 * W:4 * W].rearrange("p (s i) -> p s i", i=W),
                        st["ebT"][:, 4 * g + 2:4 * g + 4, :W],
                        mybir.AluOpType.mult,
                    )
                    st["ptrs"][g] = ptr

                def emit_avden(b_, g):
                    # den for the kv-group pair (grp = g//2) lands in one
                    # [128, 4W] tile -- g even in rows 0-63, g odd in
                    # 64-127 -- so ONE full-width reciprocal and ONE
                    # full-width normalize serve both groups (DVE time
                    # scales with elems/partition, not partitions).
                    st = blocks[b_]
                    W = st["W"]
                    grp = g // 2
                    if grp not in st["ot_g"]:
                        st["ot_g"][grp] = pot.tile([P, 4 * BQ], f32, tag="ot",
                                                   name="ot_g")
                    ptr = st["ptrs"][g]
                    hb = (g % 2) * 64
                    nc.tensor.matmul(
                        st["ot_g"][grp][hb:hb + 64, :4 * W],
                        vbs[b_][:, g * 64:(g + 1) * 64],
                        ptr[:, :4 * W],
                        start=True, stop=True,
                    )
                    if g % 2 == 0:
                        st["dens"][grp] = psc.tile([P, 4 * BQ], f32, tag="sc",
                                                   name="den")
                    den = st["dens"][grp]
                    nc.tensor.matmul(
                        den[hb:hb + 64, :4 * W], ones64[:], ptr[:, :4 * W],
                        start=True, stop=True,
                    )
                    if g % 2 == 1:
                        rcp = sp.tile([P, 4 * BQ], f32, tag="rcp", name="rcp")
                        nc.vector.reciprocal_approx_fast(
                            rcp[:, :4 * W], den[:, :4 * W])
                        st["rcps"][grp] = rcp

                def emit_norm(b_, grp, half=None):
                    # grp 0 = kv-groups 0,1 (OT slots 0,2,4,6); grp 1 = 2,3.
                    # half=0/1 emits only the lower/upper two head-slots (the
                    # last block splits so the final O-proj chain can start
                    # after the first half).
                    st = blocks[b_]
                    qs, W = st["qs"], st["W"]
                    j0, j1 = (0, 4) if half is None else (2 * half, 2 * half + 2)
                    nc.vector.tensor_tensor(
                        OT[:].rearrange("p (j s) l -> p s j l", s=2)[
                            :, grp, j0:j1, qs:qs + W],
                        st["ot_g"][grp][:, j0 * W:j1 * W].rearrange(
                            "p (s i) -> p s i", i=W),
                        st["rcps"][grp][:, j0 * W:j1 * W].rearrange(
                            "p (s i) -> p s i", i=W),
                        mybir.AluOpType.mult,
                    )

                def emit_oproj(c_, chans=(0, 1)):
                    cs = c_ * P
                    for ch in chans:
                        y_ps = pp.tile([P, 512], f32, tag="pp", name="y_ps")
                        for ot in range(8):
                            nc.tensor.matmul(
                                y_ps, OT[:, ot, cs:cs + P],
                                woT[:, ot, ch * 512:(ch + 1) * 512],
                                start=(ot == 0), stop=(ot == 7),
                            )
                        y_sb = sp.tile([P, 512], bf16, tag="ysb", name="ysb")
                        nc.scalar.copy(y_sb[:], y_ps[:])
                        nc.sync.dma_start(
                            out_e.ap()[cs:cs + P, ch * 512:(ch + 1) * 512],
                            y_sb[:],
                        )

                # final O-proj chunk as two half-chains: slots 0-3 fire
                # after the last block's first normalize (fills the PE while
                # the last exp chain runs), slots 4-7 + eviction after the
                # second
                cs3 = (QC - 1) * P
                y3 = [None, None]

                def emit_y3(part, ots=None, final=False):
                    # part 0 consumes the even OT slots (written by norm grp
                    # 0 = kv-groups 0,1), part 1 the odd slots; accumulation
                    # order over slots is free
                    if ots is None:
                        ots = range(part, 8, 2)
                    for ch in range(2):
                        if part == 0:
                            y3[ch] = pp.tile([P, 512], f32, tag="pp", name="y3")
                        for ot in ots:
                            nc.tensor.matmul(
                                y3[ch], OT[:, ot, cs3:cs3 + P],
                                woT[:, ot, ch * 512:(ch + 1) * 512],
                                start=(ot == 0), stop=(ot == 7),
                            )
                        if final:
                            y_sb = sp.tile([P, 512], bf16, tag="ysb", name="ysb")
                            nc.vector.tensor_copy(y_sb[:], y3[ch][:])
                            nc.sync.dma_start(
                                out_e.ap()[cs3:cs3 + P, ch * 512:(ch + 1) * 512],
                                y_sb[:],
                            )

                QS = [(b, g) for b in range(NB) for g in range(4)]
                NS = len(QS)
                for c_ in range(2):
                    emit_qtrans(c_, 0)
                    emit_qtrans(c_, 1)
                emit_vproj(0)
                emit_vproj(1)
                emit_scores(*QS[0])
                emit_expmult(*QS[0])
                emit_scores(*QS[1])
                emit_expmult(*QS[1])
                emit_scores(*QS[2])
                emit_expmult(*QS[2])
                for i in range(NS):
                    b_, _q = QS[i]
                    emit_avden(*QS[i])
                    if i + 3 < NS:
                        emit_scores(*QS[i + 3])
                        emit_expmult(*QS[i + 3])
                    j = i % 4
                    if j == 0 and b_ + 2 <= QC - 1:
                        emit_qtrans(b_ + 2, 0)
                        emit_qtrans(b_ + 2, 1)
                    if j == 1:
                        if b_ + 1 < NB - 1:
                            emit_vproj(b_ + 2)
                        emit_norm(b_, 0)
                        if b_ == NB - 1:
                            emit_y3(0)
                    elif j == 3:
                        if b_ == NB - 1:
                            emit_norm(b_, 1, half=0)
                            emit_y3(1, ots=(1, 3))
                            emit_norm(b_, 1, half=1)
                            emit_y3(1, ots=(5, 7), final=True)
                        else:
                            emit_norm(b_, 1)
                            if b_ >= 1:
                                emit_oproj(b_ - 1)
    nc.compile()
    return nc


def _shard_inputs(x, wq, wk, wv, wo, q_norm_w, k_norm_w):
    u = (np.asarray(q_norm_w, np.float32) * np.asarray(k_norm_w, np.float32))

    def ktile(wT):  # [DIM, O] -> [128, 8, O] bf16 (k-tiled)
        return np.ascontiguousarray(
            wT.astype(BF16).reshape(8, P, -1).transpose(1, 0, 2))

    wqT = ktile(np.asarray(wq, np.float32).T)
    wkT = ktile(np.asarray(wk, np.float32).T)
    wvT = ktile(np.asarray(wv, np.float32).T)
    woT = ktile(np.asarray(wo, np.float32).T)  # wo[e, o] -> [o, e], contraction o
    uvec = np.tile(u, 2).reshape(P, 1).astype(np.float32)
    ebTr = _exp_bias_t(edge=False)
    ident = np.eye(P, dtype=np.float32).astype(BF16)

    in_maps = []
    for c in range(8):
        b, j = c // 4, c % 4
        xh = np.zeros((LX, DIM), np.float32)
        lo = j * LQ - HALO
        s0, s1 = max(lo, 0), min(j * LQ + NB * BQ, L)
        xh[s0 - lo:s1 - lo] = x[b, s0:s1]
        xtc = np.ascontiguousarray(
            xh.T.astype(BF16).reshape(8, P, LX).transpose(1, 0, 2))
        ebT0 = _exp_bias_t(edge=(j == 0))
        in_maps.append({
            "xt": xtc, "wqT": wqT, "wkT": wkT, "wvT": wvT, "woT": woT,
            "ebT0": ebT0, "ebTr": ebTr, "ident": ident, "uvec": uvec,
        })
    return in_maps


def _run(inputs, trace=False):
    global _compiled
    in_maps = _shard_inputs(**inputs)
    if _compiled is None:
        _compiled = _build()
    nc = _compiled
    res = run_bass_kernel_spmd(nc, in_maps, list(range(8)), trace=trace)
    full = np.empty((B, L, DIM), np.float32)
    for c in range(8):
        b, j = c // 4, c % 4
        full[b, j * LQ:(j + 1) * LQ] = res.results[c]["out"].astype(np.float32)
    return full, res


def kernel(x, wq, wk, wv, wo, q_norm_w, k_norm_w):
    full, _ = _run(dict(x=np.asarray(x), wq=np.asarray(wq), wk=np.asarray(wk),
                        wv=np.asarray(wv), wo=np.asarray(wo),
                        q_norm_w=np.asarray(q_norm_w),
                        k_norm_w=np.asarray(k_norm_w)))
    return full


# revision 64
# speedup vs baseline: 1.2290x; 1.0078x over previous
"""ALiBi sliding-window GQA attention on 8 Trainium2 NeuronCores.

Sharding: batch (2) x sequence quarter (4) -> 8 cores, each computing a
disjoint [512, 1024] output chunk from a 528-token input slice (16-token
halo on the left for the sliding window). No collectives needed.

Per-core kernel (bf16 compute, f32 accumulate), v3 dataflow:
  1. K projection runs weights-stationary so K^T materializes directly in
     [feature, token] layout (no PE transposes), accumulating per kt-tile
     as the xt DMA chunks stream in -- the PE starts ~5us earlier than a
     monolithic load allows.  K is NOT normalized in SBUF: its RMSNorm
     rstd (x0.125 score scale) is applied per-key-partition via the exp
     activation's scale vector, and q_norm_w*k_norm_w is folded into the
     PSUM->SBUF eviction scale.  Per-head sum-of-squares stats come from
     a block-diagonal ones matmul on the squared K^T.
  2. Q projection in [token, feature] layout (stats need free-axis
     reduction), RMSNorm'd, then transposed to [feature, token] via PE
     transpose-mode.
  3. 5 query blocks of 112 with 128-key windows; per kv-group scores come
     out PRE-TRANSPOSED: S.T[key, head-slot, query] = K^T.T @ Q^T.
       P.T = exp(rstd_k[key] * S.T) * ebT (host table: ALiBi slopes +
       causal/window mask as multiplicative zeros, pre-transposed,
       slot-permuted)
     row sums via ones-matmul, paired so one full-width reciprocal and
     one full-width normalize serve two kv-groups; normalization fused
     into the AV PSUM->SBUF eviction.  exp on Scalar, ebT-mult split
     GpSimd/Vector.
  4. Output projection uses OUT^T as the stationary operand so results
     land in [token, feature] layout for contiguous stores.
"""

import math

import numpy as np
import ml_dtypes

import concourse.bass as bass
import concourse.tile as tile
from concourse import bacc, mybir
from concourse.bass_utils import run_bass_kernel_spmd

BF16 = ml_dtypes.bfloat16

B, L, DIM = 2, 2048, 1024
N_HEADS, N_KV_HEADS, HEAD_DIM = 16, 4, 64
WINDOW = 16
EPS = 0.01

LQ = 512           # queries per core
HALO = WINDOW      # left halo
NB = 5             # query blocks per core (attention)
BQ = 112           # queries per block
BK = 128           # key window per block
LX = HALO + NB * BQ  # 576 = padded x slice width per core
LH = LX // 2       # 288: K-proj moving-operand half
P = 128
QC = 4             # Q/O projection chunks of 128 tokens (4*128 = LQ exactly)

# head h = g + 4j (g = kv group) -> scores slot 4g + j: each attention step
# batches the 4 heads of one kv group into a single matmul
SLOT = [4 * (h % 4) + h // 4 for h in range(N_HEADS)]

_compiled = None


def _alibi_slopes(n_heads):
    closest = 2 ** math.floor(math.log2(n_heads))
    base = 2.0 ** (-(2.0 ** (-(math.log2(closest) - 3))))
    slopes = base ** np.arange(1, closest + 1, dtype=np.float64)
    if closest < n_heads:
        eb = 2.0 ** (-(2.0 ** (-(math.log2(2 * closest) - 3))))
        extra = eb ** np.arange(1, 2 * (n_heads - closest) + 1, 2, dtype=np.float64)
        slopes = np.concatenate([slopes, extra])
    return slopes[:n_heads]


def _exp_bias_t(edge: bool) -> np.ndarray:
    """[BK, N_HEADS, BQ] transposed multiplicative softmax bias, slot order.

    Query i (block-local) sits at window column jk in [i, i+16]; entry is
    exp(slope_h * (jk - 16 - i)) inside the band, 0 outside.  With
    edge=True (first block of the sequence) keys at global position < 0
    (jk < 16) are additionally masked.
    """
    slopes = _alibi_slopes(N_HEADS)
    i = np.arange(BQ)[:, None]
    jk = np.arange(BK)[None, :]
    rel = jk - WINDOW - i                      # [BQ, BK]
    valid = (rel <= 0) & (rel >= -WINDOW)
    if edge:
        valid = valid & (jk >= WINDOW)
    arg = np.where(valid[None], slopes[:, None, None] * rel[None], -np.inf)
    eb = np.exp(arg)                           # [H, BQ, BK]
    perm = np.empty(N_HEADS, np.int64)
    perm[SLOT] = np.arange(N_HEADS)            # slot s holds head perm[s]
    return np.ascontiguousarray(eb[perm].transpose(2, 0, 1)).astype(BF16)


def _build():
    nc = bacc.Bacc("TRN2", target_bir_lowering=False, debug=False)
    f32, bf16 = mybir.dt.float32, mybir.dt.bfloat16

    xt_e = nc.dram_tensor("xt", [P, 8, LX], bf16, kind="ExternalInput")
    wq_e = nc.dram_tensor("wqT", [P, 8, 1024], bf16, kind="ExternalInput")
    wk_e = nc.dram_tensor("wkT", [P, 8, 256], bf16, kind="ExternalInput")
    wv_e = nc.dram_tensor("wvT", [P, 8, 256], bf16, kind="ExternalInput")
    wo_e = nc.dram_tensor("woT", [P, 8, 1024], bf16, kind="ExternalInput")
    eb0_e = nc.dram_tensor("ebT0", [BK, N_HEADS, BQ], bf16, kind="ExternalInput")
    ebr_e = nc.dram_tensor("ebTr", [BK, N_HEADS, BQ], bf16, kind="ExternalInput")
    id_e = nc.dram_tensor("ident", [P, P], bf16, kind="ExternalInput")
    u_e = nc.dram_tensor("uvec", [P, 1], f32, kind="ExternalInput")
    out_e = nc.dram_tensor("out", [LQ, DIM], bf16, kind="ExternalOutput")

    NQ = 2 * QC  # 8 Q-chunk tiles

    with tile.TileContext(nc) as tc:
        with (
            tc.tile_pool(name="w", bufs=1) as wp,
            tc.tile_pool(name="glob", bufs=1) as gp,
            tc.tile_pool(name="raw", bufs=NQ) as rp,
            tc.tile_pool(name="stage", bufs=4) as sp,
            tc.tile_pool(name="small", bufs=NQ) as mp,
            tc.tile_pool(name="att", bufs=3) as ap,
            tc.tile_pool(name="ptrs", bufs=2 * NB) as pt,
            tc.tile_pool(name="vpool", bufs=NB) as vp,
            tc.tile_pool(name="pp", bufs=2, space="PSUM") as pp,
        ):
            # ---- PE warmup: junk matmuls bridge the DMA-bound prologue so
            # the HAM clock-gate opens before real work arrives ----
            junk = wp.tile([P, 512], bf16)
            nc.vector.memset(junk[:], 1.0)
            ones64 = wp.tile([P, 64], bf16)
            nc.vector.memset(ones64[:], 1.0)
            epsv = wp.tile([P, 1], f32)
            nc.vector.memset(epsv[:], EPS)
            eps64v = wp.tile([P, 1], f32)
            nc.vector.memset(eps64v[:], HEAD_DIM * EPS)
            # block-diagonal ones: col j = 1 on partitions [64j, 64j+64)
            ones_blk = wp.tile([P, 2], bf16)
            nc.vector.memset(ones_blk[:], 0.0)
            nc.vector.memset(ones_blk[:64, 0:1], 1.0)
            nc.vector.memset(ones_blk[64:, 1:2], 1.0)
            wps = pp.tile([P, 512], f32, tag="pp", name="warm")[:64]
            for _ in range(10):
                nc.tensor.matmul(wps, ones64[:], junk[:], start=True, stop=True)

            # ---- input loads; wk first, then xt kt-pair by kt-pair so the
            # K projection starts accumulating as chunks land ----
            xt = wp.tile([P, 8, LX], bf16)
            wkT = wp.tile([P, 8, 256], bf16)
            wqT = wp.tile([P, 8, 1024], bf16)
            wvT = wp.tile([P, 8, 256], bf16)
            woT = wp.tile([P, 8, 1024], bf16)
            uvec = wp.tile([P, 1], f32)
            ebT0 = wp.tile([BK, N_HEADS, BQ], bf16)
            ebTr = wp.tile([BK, N_HEADS, BQ], bf16)
            ident = wp.tile([P, P], bf16)
            nc.sync.dma_start(wkT[:], wk_e.ap())
            for g in range(4):
                nc.sync.dma_start(xt[:, 2 * g:2 * g + 2],
                                  xt_e.ap()[:, 2 * g:2 * g + 2])
            nc.sync.dma_start(uvec[:], u_e.ap())
            for g in range(4):
                nc.sync.dma_start(wqT[:, 2 * g:2 * g + 2],
                                  wq_e.ap()[:, 2 * g:2 * g + 2])
            nc.sync.dma_start(wvT[:], wv_e.ap())
            nc.sync.dma_start(ebT0[:], eb0_e.ap())
            nc.sync.dma_start(ebTr[:], ebr_e.ap())
            nc.sync.dma_start(ident[:], id_e.ap())
            for g in range(4):
                nc.sync.dma_start(woT[:, 2 * g:2 * g + 2],
                                  wo_e.ap()[:, 2 * g:2 * g + 2])

            QT = gp.tile([P, 8, LQ], bf16)    # normalized Q transposed
            KT = gp.tile([P, 2, LX], bf16)    # raw K^T, u-scaled
            sqKT = gp.tile([P, 2, LX], bf16)  # squared raw K^T (stats)
            OT = gp.tile([P, 8, LQ], bf16)    # attention out transposed

            vbs = []
            rks = []
            with (
                tc.tile_pool(name="kacc", bufs=4, space="PSUM") as ka,
                tc.tile_pool(name="kst", bufs=2, space="PSUM") as ks,
            ):
                # ---- phase 1a: K projection, weights-stationary ----
                kaccs = [ka.tile([P, 512], f32, tag="ka", name="kacc")[:, :LH]
                         for _ in range(4)]
                for kt in range(8):
                    for fh in range(2):
                        for lh in range(2):
                            nc.tensor.matmul(
                                kaccs[2 * fh + lh],
                                wkT[:, kt, fh * P:(fh + 1) * P],
                                xt[:, kt, lh * LH:(lh + 1) * LH],
                                start=(kt == 0), stop=(kt == 7),
                            )
                for fh in range(2):
                    for lh in range(2):
                        acc = kaccs[2 * fh + lh]
                        nc.scalar.activation(
                            KT[:, fh, lh * LH:(lh + 1) * LH], acc[:],
                            mybir.ActivationFunctionType.Copy, scale=uvec[:],
                        )
                        nc.scalar.square(
                            sqKT[:, fh, lh * LH:(lh + 1) * LH], acc[:])

                # ---- phase 1c: K RMSNorm stats per key window ----
                # rk = 0.125 / sqrt(mean(k^2) + eps) = 1/sqrt(sumsq + 64*eps)
                # per (window key partition, kv-head); consumed as the exp
                # activation's per-partition scale vector.
                for b_ in range(NB):
                    qs = b_ * BQ
                    kss = ks.tile([P, 4], f32, tag="ks", name="kss")
                    for fh in range(2):
                        nc.tensor.matmul(
                            kss[:, 2 * fh:2 * fh + 2],
                            sqKT[:, fh, qs:qs + BK],
                            ones_blk[:],
                            start=True, stop=True,
                        )
                    srtk = mp.tile([P, 8], f32, tag="srt", name="srtk")[:, :4]
                    nc.scalar.activation(
                        srtk[:], kss[:], mybir.ActivationFunctionType.Sqrt,
                        scale=1.0, bias=eps64v[:],
                    )
                    rk = mp.tile([P, 4], f32, tag="rk", name="rk", bufs=NB)
                    nc.vector.reciprocal(rk[:], srtk[:])
                    rks.append(rk)

            with (
                tc.tile_pool(name="pot", bufs=2, space="PSUM") as pot,
                tc.tile_pool(name="psc", bufs=4, space="PSUM") as psc,
            ):
                # ---- phase 2: Q projection -> raw [token, feature] ----
                raws = []
                for c_ in range(QC):
                    cs = c_ * P
                    for ch in range(2):
                        q_ps = pp.tile([P, 512], f32, tag="pp", name="q_ps")
                        for kt in range(8):
                            nc.tensor.matmul(
                                q_ps,
                                xt[:, kt, HALO + cs:HALO + cs + P],
                                wqT[:, kt, ch * 512:(ch + 1) * 512],
                                start=(kt == 0), stop=(kt == 7),
                            )
                        raw = rp.tile([P, 512], bf16, tag="raw", name="q_raw")
                        nc.scalar.copy(raw[:], q_ps[:])
                        raws.append(raw)

                # ---- Q RMSNorm, per-chain so each hat completes ~2.5us
                # after its projection chain (Copy/Square/Sqrt share one
                # ACT table set, so no LUT thrash) ----
                hats = []
                for raw in raws:
                    sq = sp.tile([P, 512], bf16, tag="sq", name="sq")
                    nc.scalar.square(sq[:], raw[:])
                    ss = mp.tile([P, 8], f32, tag="ss", name="ss")
                    nc.vector.reduce_sum(
                        ss[:], sq[:].rearrange("l (h d) -> l h d", d=HEAD_DIM),
                        axis=mybir.AxisListType.X,
                    )
                    srt = mp.tile([P, 8], f32, tag="srt", name="srt")
                    nc.scalar.activation(
                        srt[:], ss[:], mybir.ActivationFunctionType.Sqrt,
                        scale=1.0 / HEAD_DIM, bias=epsv[:],
                    )
                    rstd = mp.tile([P, 8], f32, tag="rstd", name="rstd")
                    nc.vector.reciprocal(rstd[:], srt[:])
                    hat = rp.tile([P, 512], bf16, tag="hat", name="hat")
                    nc.vector.tensor_tensor(
                        hat[:].rearrange("l (h d) -> l h d", d=HEAD_DIM),
                        raw[:].rearrange("l (h d) -> l h d", d=HEAD_DIM),
                        rstd[:, :, None].to_broadcast((P, 8, HEAD_DIM)),
                        mybir.AluOpType.mult,
                    )
                    hats.append(hat)

                # dummy exp: forces the Exp<->Sqrt ACT-table swap to happen
                # here (during the transpose phase) instead of on the first
                # real exp's critical path
                dume = mp.tile([P, 8], f32, tag="srt", name="dume")
                nc.scalar.activation(
                    dume[:, :1], epsv[:], mybir.ActivationFunctionType.Exp)

                def emit_qtrans(c_, ch):
                    cs = c_ * P
                    hat = hats[2 * c_ + ch]
                    tp = pp.tile([P, 4, P], bf16, tag="pp", name="tpq")
                    for ot in range(4):
                        nc.tensor.transpose(
                            tp[:, ot], hat[:, ot * P:(ot + 1) * P], ident[:])
                    dst = QT[:, ch * 4:ch * 4 + 4, cs:cs + P]
                    if ch % 2 == 0:
                        nc.vector.tensor_copy(dst, tp[:])
                    else:
                        nc.scalar.copy(dst, tp[:])

                # ---- phase 4: attention + output projection ----
                # Flat software-pipelined stream over NB*4 kv-group steps.
                # All 4 heads of kv-group g share the same K/V stationary
                # operand, so each stage is ONE matmul with the 4 heads
                # batched in the moving operand (448 cols).  Head h = g + 4j
                # lives at score-slot (g, j); packed flat [4W] in PSUM.  The
                # PE queue runs scores(i+2) between scores(i) and AV(i) so
                # the exp -> ebT-mult chain latency is hidden by real matmul
                # work even across block boundaries; O-projection chunks
                # slot in at block boundaries as extra filler.
                blocks = []
                for b_ in range(NB):
                    blocks.append(dict(
                        qs=b_ * BQ,
                        W=BQ if b_ < NB - 1 else LQ - (NB - 1) * BQ,
                        ebT=ebT0 if b_ == 0 else ebTr,
                        rk=rks[b_],
                        ot_g={}, rcps={}, dens={}, scs={}, ptrs={},
                    ))

                def emit_vproj(b_):
                    # V projection for block b_'s key window, deferred into
                    # the attention stream as PE filler (vb is first needed
                    # at block b_'s AV step)
                    qs = b_ * BQ
                    v_ps = pp.tile([P, 512], f32, tag="pp", name="v_ps")[:, :256]
                    for kt in range(8):
                        nc.tensor.matmul(
                            v_ps, xt[:, kt, qs:qs + BK], wvT[:, kt],
                            start=(kt == 0), stop=(kt == 7),
                        )
                    vb = vp.tile([P, 256], bf16, tag="vb", name="vb")
                    nc.scalar.copy(vb[:], v_ps[:])
                    vbs.append(vb)

                def emit_scores(b_, g):
                    st = blocks[b_]
                    qs, W = st["qs"], st["W"]
                    sc = psc.tile([P, 4 * BQ], f32, tag="sc", name="sc")
                    # moving: the 4 heads {g+4j} = OT-slots g//2 + 2j of QT
                    mv = QT[:].rearrange("p (j s) l -> p s j l", s=2)[
                        (g % 2) * 64:(g % 2) * 64 + 64, g // 2, :, qs:qs + W]
                    nc.tensor.matmul(
                        sc[:, :4 * W],
                        KT[(g % 2) * 64:(g % 2) * 64 + 64, g // 2, qs:qs + BK],
                        mv,
                        start=True, stop=True,
                    )
                    st["scs"][g] = sc

                def emit_expmult(b_, g):
                    st = blocks[b_]
                    W = st["W"]
                    e_t = ap.tile([P, 4 * BQ], bf16, tag="et", name="e_t")
                    nc.scalar.activation(
                        e_t[:, :4 * W], st["scs"][g][:, :4 * W],
                        mybir.ActivationFunctionType.Exp,
                        scale=st["rk"][:, g:g + 1],
                    )
                    # mask+ALiBi multiply split across GpSimd (slots 0-1)
                    # and Vector (slots 2-3)
                    ptr = pt.tile([P, 4 * BQ], bf16, tag="ptr", name="ptr")
                    nc.gpsimd.tensor_tensor(
                        ptr[:, :2 * W].rearrange("p (s i) -> p s i", i=W),
                        e_t[:, :2 * W].rearrange("p (s i) -> p s i", i=W),
                        st["ebT"][:, 4 * g:4 * g + 2, :W],
                        mybir.AluOpType.mult,
                    )
                    nc.vector.tensor_tensor(
                        ptr[:, 2 * W:4 * W].rearrange("p (s i) -> p s i", i=W),
                        e_t[:, 2 * W:4 * W].rearrange("p (s i) -> p s i", i=W),
                        st["ebT"][:, 4 * g + 2:4 * g + 4, :W],
                        mybir.AluOpType.mult,
                    )
                    st["ptrs"][g] = ptr

                def emit_avden(b_, g):
                    # den for the kv-group pair (grp = g//2) lands in one
                    # [128, 4W] tile -- g even in rows 0-63, g odd in
                    # 64-127 -- so ONE full-width reciprocal and ONE
                    # full-width normalize serve both groups (DVE time
                    # scales with elems/partition, not partitions).
                    st = blocks[b_]
                    W = st["W"]
                    grp = g // 2
                    if grp not in st["ot_g"]:
                        st["ot_g"][grp] = pot.tile([P, 4 * BQ], f32, tag="ot",
                                                   name="ot_g")
                    ptr = st["ptrs"][g]
                    hb = (g % 2) * 64
                    nc.tensor.matmul(
                        st["ot_g"][grp][hb:hb + 64, :4 * W],
                        vbs[b_][:, g * 64:(g + 1) * 64],
                        ptr[:, :4 * W],
                        start=True, stop=True,
                    )
                    if g % 2 == 0:
                        st["dens"][grp] = psc.tile([P, 4 * BQ], f32, tag="sc",
                                                   name="den")
                    den = st["dens"][grp]
                    nc.tensor.matmul(
                        den[hb:hb + 64, :4 * W], ones64[:], ptr[:, :4 * W],
                        start=True, stop=True,
                    )
                    if g % 2 == 1:
                        rcp = sp.tile([P, 4 * BQ], f32, tag="rcp", name="rcp")
                        nc.vector.reciprocal_approx_fast(
                            rcp[:, :4 * W], den[:, :4 * W])
                        st["rcps"][grp] = rcp

                def emit_norm(b_, grp, half=None):
                    # grp 0 = kv-groups 0,1 (OT slots 0,2,4,6); grp 1 = 2,3.
                    # half=0/1 emits only the lower/upper two head-slots (the
                    # last block splits so the final O-proj chain can start
                    # after the first half).
                    st = blocks[b_]
                    qs, W = st["qs"], st["W"]
                    j0, j1 = (0, 4) if half is None else (2 * half, 2 * half + 2)
                    nc.vector.tensor_tensor(
                        OT[:].rearrange("p (j s) l -> p s j l", s=2)[
                            :, grp, j0:j1, qs:qs + W],
                        st["ot_g"][grp][:, j0 * W:j1 * W].rearrange(
                            "p (s i) -> p s i", i=W),
                        st["rcps"][grp][:, j0 * W:j1 * W].rearrange(
                            "p (s i) -> p s i", i=W),
                        mybir.AluOpType.mult,
                    )

                def emit_oproj(c_, chans=(0, 1)):
                    cs = c_ * P
                    for ch in chans:
                        y_ps = pp.tile([P, 512], f32, tag="pp", name="y_ps")
                        for ot in range(8):
                            nc.tensor.matmul(
                                y_ps, OT[:, ot, cs:cs + P],
                                woT[:, ot, ch * 512:(ch + 1) * 512],
                                start=(ot == 0), stop=(ot == 7),
                            )
                        y_sb = sp.tile([P, 512], bf16, tag="ysb", name="ysb")
                        nc.scalar.copy(y_sb[:], y_ps[:])
                        nc.sync.dma_start(
                            out_e.ap()[cs:cs + P, ch * 512:(ch + 1) * 512],
                            y_sb[:],
                        )

                # final O-proj chunk as two half-chains: slots 0-3 fire
                # after the last block's first normalize (fills the PE while
                # the last exp chain runs), slots 4-7 + eviction after the
                # second
                cs3 = (QC - 1) * P
                y3 = [None, None]

                def emit_y3(part, ots=None, final=False):
                    # part 0 consumes the even OT slots (written by norm grp
                    # 0 = kv-groups 0,1), part 1 the odd slots; accumulation
                    # order over slots is free
                    if ots is None:
                        ots = range(part, 8, 2)
                    for ch in range(2):
                        if part == 0:
                            y3[ch] = pp.tile([P, 512], f32, tag="pp", name="y3")
                        for ot in ots:
                            nc.tensor.matmul(
                                y3[ch], OT[:, ot, cs3:cs3 + P],
                                woT[:, ot, ch * 512:(ch + 1) * 512],
                                start=(ot == 0), stop=(ot == 7),
                            )
                        if final:
                            y_sb = sp.tile([P, 512], bf16, tag="ysb", name="ysb")
                            nc.vector.tensor_copy(y_sb[:], y3[ch][:])
                            nc.sync.dma_start(
                                out_e.ap()[cs3:cs3 + P, ch * 512:(ch + 1) * 512],
                                y_sb[:],
                            )

                QS = [(b, g) for b in range(NB) for g in range(4)]
                NS = len(QS)
                # all V projections run here: their inputs are long since
                # ready, so they fill the PE while the Q-norm chain
                # (hats -> transposes) drains on Scalar/Vector
                for b2 in range(NB):
                    emit_vproj(b2)
                for c_ in range(2):
                    emit_qtrans(c_, 0)
                    emit_qtrans(c_, 1)
                emit_scores(*QS[0])
                emit_expmult(*QS[0])
                emit_scores(*QS[1])
                emit_expmult(*QS[1])
                emit_scores(*QS[2])
                emit_expmult(*QS[2])
                for i in range(NS):
                    b_, _q = QS[i]
                    emit_avden(*QS[i])
                    if i + 3 < NS:
                        emit_scores(*QS[i + 3])
                        emit_expmult(*QS[i + 3])
                    j = i % 4
                    if j == 0 and b_ + 2 <= QC - 1:
                        emit_qtrans(b_ + 2, 0)
                        emit_qtrans(b_ + 2, 1)
                    if j == 1:
                        emit_norm(b_, 0)
                        if b_ == NB - 1:
                            emit_y3(0)
                    elif j == 3:
                        if b_ == NB - 1:
                            emit_norm(b_, 1, half=0)
                            emit_y3(1, ots=(1, 3))
                            emit_norm(b_, 1, half=1)
                            emit_y3(1, ots=(5, 7), final=True)
                        else:
                            emit_norm(b_, 1)
                            if b_ >= 1:
                                emit_oproj(b_ - 1)
    nc.compile()
    return nc


def _shard_inputs(x, wq, wk, wv, wo, q_norm_w, k_norm_w):
    u = (np.asarray(q_norm_w, np.float32) * np.asarray(k_norm_w, np.float32))

    def ktile(wT):  # [DIM, O] -> [128, 8, O] bf16 (k-tiled)
        return np.ascontiguousarray(
            wT.astype(BF16).reshape(8, P, -1).transpose(1, 0, 2))

    wqT = ktile(np.asarray(wq, np.float32).T)
    wkT = ktile(np.asarray(wk, np.float32).T)
    wvT = ktile(np.asarray(wv, np.float32).T)
    woT = ktile(np.asarray(wo, np.float32).T)  # wo[e, o] -> [o, e], contraction o
    uvec = np.tile(u, 2).reshape(P, 1).astype(np.float32)
    ebTr = _exp_bias_t(edge=False)
    ident = np.eye(P, dtype=np.float32).astype(BF16)

    in_maps = []
    for c in range(8):
        b, j = c // 4, c % 4
        xh = np.zeros((LX, DIM), np.float32)
        lo = j * LQ - HALO
        s0, s1 = max(lo, 0), min(j * LQ + NB * BQ, L)
        xh[s0 - lo:s1 - lo] = x[b, s0:s1]
        xtc = np.ascontiguousarray(
            xh.T.astype(BF16).reshape(8, P, LX).transpose(1, 0, 2))
        ebT0 = _exp_bias_t(edge=(j == 0))
        in_maps.append({
            "xt": xtc, "wqT": wqT, "wkT": wkT, "wvT": wvT, "woT": woT,
            "ebT0": ebT0, "ebTr": ebTr, "ident": ident, "uvec": uvec,
        })
    return in_maps


def _run(inputs, trace=False):
    global _compiled
    in_maps = _shard_inputs(**inputs)
    if _compiled is None:
        _compiled = _build()
    nc = _compiled
    res = run_bass_kernel_spmd(nc, in_maps, list(range(8)), trace=trace)
    full = np.empty((B, L, DIM), np.float32)
    for c in range(8):
        b, j = c // 4, c % 4
        full[b, j * LQ:(j + 1) * LQ] = res.results[c]["out"].astype(np.float32)
    return full, res


def kernel(x, wq, wk, wv, wo, q_norm_w, k_norm_w):
    full, _ = _run(dict(x=np.asarray(x), wq=np.asarray(wq), wk=np.asarray(wk),
                        wv=np.asarray(wv), wo=np.asarray(wo),
                        q_norm_w=np.asarray(q_norm_w),
                        k_norm_w=np.asarray(k_norm_w)))
    return full
